# revision 1
# baseline (speedup 1.0000x reference)
"""GCN layer (X @ W, then COO spmm scatter-add by dest, + bias) on 8 trn2 cores.

Strategy (dest-sharded, per sharding hint):
  Launch 1 (SPMD): core c computes support shard = X[c*12500:(c+1)*12500] @ W.
    Host pre-transposes X so the contraction dim lands on partitions.
  Host: assembles full support; partitions each core's edges by destination
    into groups of 32 dests (640 edge slots each, 5 tiles of 128); groups of
    66 form a "region" whose referenced source rows are compacted into a
    <32768-row halo table (so dma_gather's int16 indices can address it).
    Builds one-hot*val scatter matrices S per 128-edge tile.
  Launch 2 (SPMD): per gather-op (11 groups = 7040 edge slots): dma_gather
    512B support rows from the region halo table -> [128 edges, 128 feats]
    tiles; PE matmul  G.T @ S  accumulates out^T[128 feats, 32 dests] in
    PSUM (fuses the val multiply and the segment sum); bias added during
    PSUM evac; out^T written to DRAM. Host transposes/concats shards.
"""

import numpy as np

import concourse.bass as bass
import concourse.tile as tile
from concourse import bacc, mybir
from concourse.bass_utils import run_bass_kernel_spmd

# ---------------- problem constants (hardcoded; kernel.py is self-contained)
N_NODES = 100000
N_EDGES = 1600000
IN_F = 256
OUT_F = 128
NCORES = 8

D_PER_CORE = N_NODES // NCORES  # 12500 dest nodes per core

# launch-1 (support matmul) geometry
ROWS_PAD = 12544  # 98 * 128

# launch-2 (gather + spmm) geometry
W_G = 32            # dests per group
CAP = 640           # edge-slot capacity per group (5 tiles of 128)
TPG = CAP // 128    # tiles per group = 5
R_GROUPS = 66       # groups per region
NREG = 6            # regions per core; 6*66=396 groups >= ceil(12500/32)=391
NGROUPS = NREG * R_GROUPS          # 396
TABLE_ROWS = 31744                 # halo-table rows per region (< 32768 for int16)
OP_GROUPS = 3                      # groups per gather op (small: SWDGE ring limit)
OPS_PER_REG = R_GROUPS // OP_GROUPS  # 22
NOPS = NREG * OPS_PER_REG          # 36 gather ops per core
IDX_PER_OP = OP_GROUPS * CAP       # 1920
G_IDX = 128                        # idxs per dma_gather (single tile; HW-validated max)
GPO = IDX_PER_OP // G_IDX          # gathers per op
TILES_PER_OP = IDX_PER_OP // 128   # 55
OUT_COLS = NGROUPS * W_G           # 12672 dest slots per core

FP32 = mybir.dt.float32
I16 = mybir.dt.int16


# ---------------- launch 1: support = X_shard @ W ----------------
def _new_nc():
    return bacc.Bacc("TRN2", target_bir_lowering=False, debug=False)


def build_support_program():
    nc = _new_nc()
    xt = nc.declare_dram_parameter("xt", [IN_F, ROWS_PAD], FP32, isOutput=False)
    w = nc.declare_dram_parameter("w", [IN_F, OUT_F], FP32, isOutput=False)
    sup = nc.declare_dram_parameter("sup", [ROWS_PAD, OUT_F], FP32, isOutput=True)

    with tile.TileContext(nc) as tc:
        with (
            tc.tile_pool(name="xt_pool", bufs=1) as xt_pool,
            tc.tile_pool(name="w_pool", bufs=1) as w_pool,
            tc.tile_pool(name="ev_pool", bufs=4) as ev_pool,
            tc.tile_pool(name="ps_pool", bufs=4, space="PSUM") as ps_pool,
        ):
            xt_t = xt_pool.tile([128, 2, ROWS_PAD], FP32)
            for k in range(2):
                nc.sync.dma_start(xt_t[:, k, :], xt[128 * k : 128 * (k + 1), :])
            w_t = w_pool.tile([128, 2, OUT_F], FP32)
            for k in range(2):
                nc.sync.dma_start(w_t[:, k, :], w[128 * k : 128 * (k + 1), :])

            for i in range(ROWS_PAD // 128):
                ps = ps_pool.tile([128, OUT_F], FP32, space="PSUM")
                for k in range(2):
                    nc.tensor.matmul(
                        out=ps[:],
                        lhsT=xt_t[:, k, 128 * i : 128 * (i + 1)],
                        rhs=w_t[:, k, :],
                        start=(k == 0),
                        stop=(k == 1),
                    )
                ev = ev_pool.tile([128, OUT_F], FP32)
                nc.vector.tensor_copy(ev[:], ps[:])
                nc.sync.dma_start(sup[128 * i : 128 * (i + 1), :], ev[:])
    nc.compile()
    return nc


# ---------------- launch 2: gather + S-matmul + bias ----------------
def build_spmm_program(n_ops=NOPS, use_gather=True):
    nc = _new_nc()
    tables = nc.declare_dram_parameter(
        "tables", [NREG, TABLE_ROWS, OUT_F], FP32, isOutput=False
    )
    idx = nc.declare_dram_parameter(
        "idx", [NOPS, 128, GPO, G_IDX // 16], I16, isOutput=False
    )
    smat = nc.declare_dram_parameter(
        "smat", [NOPS, 128, TILES_PER_OP, W_G], FP32, isOutput=False
    )
    bias = nc.declare_dram_parameter("bias", [OUT_F, 1], FP32, isOutput=False)
    out = nc.declare_dram_parameter("out", [OUT_F, OUT_COLS], FP32, isOutput=True)

    with tile.TileContext(nc) as tc:
        with (
            tc.tile_pool(name="bias_pool", bufs=1) as bias_pool,
            tc.tile_pool(name="idx_pool", bufs=3) as idx_pool,
            tc.tile_pool(name="s_pool", bufs=3) as s_pool,
            tc.tile_pool(name="g_pool", bufs=3) as g_pool,
            tc.tile_pool(name="ev_pool", bufs=3) as ev_pool,
            tc.tile_pool(name="ps_pool", bufs=2, space="PSUM") as ps_pool,
        ):
            bias_t = bias_pool.tile([128, 1], FP32)
            nc.sync.dma_start(bias_t[:], bias[:, :])

            for j in range(n_ops):
                r = j // OPS_PER_REG
                idx_t = idx_pool.tile([128, GPO, G_IDX // 16], I16)
                nc.sync.dma_start(idx_t[:], idx[j])
                s_t = s_pool.tile([128, TILES_PER_OP, W_G], FP32)
                nc.sync.dma_start(s_t[:], smat[j])

                g_t = g_pool.tile([128, TILES_PER_OP, 128], FP32)
                tpg_g = G_IDX // 128
                if use_gather:
                    for k in range(GPO):
                        nc.gpsimd.dma_gather(
                            g_t[:, k * tpg_g : (k + 1) * tpg_g, :],
                            tables[r],
                            idx_t[:, k, :],
                            G_IDX,
                            G_IDX,
                            OUT_F,
                        )
                else:
                    nc.gpsimd.memset(g_t[:], 1.0)

                ps = ps_pool.tile([128, OP_GROUPS * W_G], FP32, space="PSUM")
                for t in range(TILES_PER_OP):
                    go = t // TPG
                    nc.tensor.matmul(
                        out=ps[:, W_G * go : W_G * (go + 1)],
                        lhsT=g_t[:, t, :],
                        rhs=s_t[:, t, :],
                        start=(t % TPG == 0),
                        stop=(t % TPG == TPG - 1),
                    )
                ev = ev_pool.tile([128, OP_GROUPS * W_G], FP32)
                nc.vector.tensor_scalar(
                    out=ev[:],
                    in0=ps[:],
                    scalar1=bias_t[:],
                    scalar2=None,
                    op0=mybir.AluOpType.add,
                )
                nc.sync.dma_start(
                    out[:, OP_GROUPS * W_G * j : OP_GROUPS * W_G * (j + 1)], ev[:]
                )
    nc.compile()
    return nc


# ---------------- host-side sharding / packing ----------------
def _pack_core(rows_c, cols_c, vals_c, support):
    """Build (tables, idx, smat) arrays for one core.

    rows_c: local dest ids [0, 12500); cols_c: global src ids; vals_c: f32.
    """
    g = rows_c // W_G  # group id per edge
    order = np.lexsort((cols_c, g))
    g = g[order]
    w = (rows_c % W_G)[order]
    cols_s = cols_c[order]
    vals_s = vals_c[order]

    cnt = np.bincount(g, minlength=NGROUPS)
    if cnt.max() > CAP:
        raise RuntimeError(f"group overflow: {cnt.max()} > {CAP}")

    # slot within group for each (group-sorted) edge
    starts = np.zeros(NGROUPS + 1, np.int64)
    np.cumsum(cnt, out=starts[1:])
    slot_in_group = np.arange(len(g)) - starts[g]
    slot = g.astype(np.int64) * CAP + slot_in_group  # global padded slot

    idx_all = np.zeros(NGROUPS * CAP, np.int16)  # padding -> row 0
    tables = np.zeros((NREG, TABLE_ROWS, OUT_F), np.float32)
    reg_of_edge = g // R_GROUPS
    for r in range(NREG):
        m = reg_of_edge == r
        if not m.any():
            continue
        u, inv = np.unique(cols_s[m], return_inverse=True)
        if len(u) > TABLE_ROWS:
            raise RuntimeError(f"region overflow: {len(u)} > {TABLE_ROWS}")
        tables[r, : len(u)] = support[u]
        idx_all[slot[m]] = inv.astype(np.int16)

    smat = np.zeros((NGROUPS * CAP // 128, 128, W_G), np.float32)
    smat[slot // 128, slot % 128, w] = vals_s
    smat = smat.reshape(NOPS, TILES_PER_OP, 128, W_G).transpose(0, 2, 1, 3)
    smat = np.ascontiguousarray(smat)  # [NOPS, 128, TILES_PER_OP, W_G]

    # idx wrap per gather: idx i -> partition i%16, free slot i//16; replicate x8
    idx4 = idx_all.reshape(NOPS, GPO, G_IDX // 16, 16).transpose(0, 1, 3, 2)
    idx4 = np.tile(idx4, (1, 1, 8, 1))  # [NOPS, GPO, 128, G_IDX//16]
    idx_t = np.ascontiguousarray(idx4.transpose(0, 2, 1, 3))
    return tables, idx_t, smat


def kernel(X_input, adj_row, adj_col, adj_val, W, bias):
    X_input = np.asarray(X_input, np.float32)
    adj_row = np.asarray(adj_row)
    adj_col = np.asarray(adj_col)
    adj_val = np.asarray(adj_val, np.float32)
    W = np.asarray(W, np.float32)
    bias = np.asarray(bias, np.float32)

    # ---- launch 1: support shards
    nc1 = build_support_program()
    xT = np.ascontiguousarray(X_input.T)
    in_maps1 = []
    for c in range(NCORES):
        sl = np.zeros((IN_F, ROWS_PAD), np.float32)
        lo = c * D_PER_CORE
        sl[:, :D_PER_CORE] = xT[:, lo : lo + D_PER_CORE]
        in_maps1.append({"xt": sl, "w": W})
    res1 = run_bass_kernel_spmd(nc1, in_maps1, list(range(NCORES)))
    kernel.last_res1 = res1
    support = np.concatenate(
        [res1.results[c]["sup"][:D_PER_CORE] for c in range(NCORES)], axis=0
    )  # [100000, 128]

    # ---- host packing
    core_of = adj_row // D_PER_CORE
    in_maps2 = []
    bias_col = np.ascontiguousarray(bias.reshape(OUT_F, 1))
    for c in range(NCORES):
        m = core_of == c
        tables, idx_t, smat = _pack_core(
            (adj_row[m] - c * D_PER_CORE).astype(np.int64),
            adj_col[m].astype(np.int64),
            adj_val[m],
            support,
        )
        in_maps2.append(
            {"tables": tables, "idx": idx_t, "smat": smat, "bias": bias_col}
        )

    # ---- launch 2
    nc2 = build_spmm_program()
    res2 = run_bass_kernel_spmd(nc2, in_maps2, list(range(NCORES)))
    kernel.last_res2 = res2
    out = np.empty((N_NODES, OUT_F), np.float32)
    for c in range(NCORES):
        o = res2.results[c]["out"]  # [128, OUT_COLS]
        out[c * D_PER_CORE : (c + 1) * D_PER_CORE] = o[:, :D_PER_CORE].T
    return out



# revision 9
# speedup vs baseline: 11.6907x; 11.6907x over previous
"""GCN layer (X @ W, then COO spmm scatter-add by dest, + bias) on 8 trn2 cores.

Strategy (dest-sharded, per sharding hint):
  Launch 1 (SPMD): core c computes support shard = X[c*12500:(c+1)*12500] @ W
    in bf16 (fp32 PSUM accumulate). Host pre-transposes X so the contraction
    dim lands on partitions.
  Host: assembles full support (bf16); packs each core's 12500 dest nodes
    into bins of <=32 dests and <=511 edges (next-fit-decreasing by degree).
    Per bin: a halo table T of the bin's unique source support rows (<=512
    rows, last row = bias) and a values matrix C [512, 32] with
    C[src_slot, dest_slot] = edge val (bias row = 1). out_bin = C^T @ T.
  Launch 2 (SPMD): pure sequential streaming -- no gathers. Per 64-chunk
    slab: stream T [128, 64*128] and C [128, 64*32] (host pre-swizzled so
    chunk rows land on partitions), one matmul per chunk accumulating each
    bin's 4 chunks in PSUM ([32 dests, 128 feats] per bin, 4 bins per PSUM
    bank), DVE-evacuate to bf16, DMA out per slab. Host unpermutes dest
    rows and casts to fp32.
"""

import numpy as np
import ml_dtypes

import concourse.bass as bass  # noqa: F401  (kept for parity with tile API)
import concourse.tile as tile
from concourse import bacc, mybir
from concourse.bass_utils import run_bass_kernel_spmd

BF16_NP = ml_dtypes.bfloat16

# ---------------- problem constants (hardcoded; kernel.py is self-contained)
N_NODES = 100000
N_EDGES = 1600000
IN_F = 256
OUT_F = 128
NCORES = 8

D_PER_CORE = N_NODES // NCORES  # 12500 dest nodes per core

# launch-1 (support matmul) geometry
ROWS_PAD = 12544  # 98 * 128
RTILES = ROWS_PAD // 128

# launch-2 (streamed halo spmm) geometry
W_G = 32  # dests per bin
CAP = 512  # table rows per bin (4 chunks of 128; last row = bias)
EDGE_CAP = CAP - 1  # <=511 edges -> <=511 unique sources -> row 511 free
CPB = CAP // 128  # chunks per bin = 4
SLAB_CHUNKS = 64  # chunks per slab (16 bins)
BINS_PER_SLAB = SLAB_CHUNKS // CPB  # 16
NSLABS = 25
NBINS = NSLABS * BINS_PER_SLAB  # 400
NCHUNKS = NBINS * CPB  # 1728

FP32 = mybir.dt.float32
BF16 = mybir.dt.bfloat16


def _new_nc():
    return bacc.Bacc("TRN2", target_bir_lowering=False, debug=False)


# ---------------- launch 1: support = X_shard @ W (bf16) ----------------
def build_support_program():
    nc = _new_nc()
    xt = nc.declare_dram_parameter("xt", [2, 128, ROWS_PAD], BF16, isOutput=False)
    w = nc.declare_dram_parameter("w", [2, 128, OUT_F], BF16, isOutput=False)
    sup = nc.declare_dram_parameter("sup", [128, RTILES, OUT_F], BF16, isOutput=True)

    with tile.TileContext(nc) as tc:
        with (
            tc.tile_pool(name="xt_pool", bufs=1) as xt_pool,
            tc.tile_pool(name="w_pool", bufs=1) as w_pool,
            tc.tile_pool(name="out_pool", bufs=1) as out_pool,
            tc.tile_pool(name="ps_pool", bufs=4, space="PSUM") as ps_pool,
        ):
            xt_t = xt_pool.tile([128, 2, ROWS_PAD], BF16)
            for k in range(2):
                nc.sync.dma_start(xt_t[:, k, :], xt[k])
            w_t = w_pool.tile([128, 2, OUT_F], BF16)
            for k in range(2):
                nc.sync.dma_start(w_t[:, k, :], w[k])

            sup_buf = out_pool.tile([128, RTILES, OUT_F], BF16)
            for i in range(RTILES):
                ps = ps_pool.tile([128, OUT_F], FP32, space="PSUM")
                for k in range(2):
                    nc.tensor.matmul(
                        out=ps[:],
                        lhsT=xt_t[:, k, 128 * i : 128 * (i + 1)],
                        rhs=w_t[:, k, :],
                        start=(k == 0),
                        stop=(k == 1),
                    )
                nc.vector.tensor_copy(sup_buf[:, i, :], ps[:])
            nc.sync.dma_start(sup[:], sup_buf[:])
    nc.compile()
    return nc


# ---------------- launch 2: streamed halo spmm ----------------
def build_spmm_program():
    nc = _new_nc()
    tswz = nc.declare_dram_parameter(
        "tswz", [NSLABS, 128, SLAB_CHUNKS * OUT_F], BF16, isOutput=False
    )
    cswz = nc.declare_dram_parameter(
        "cswz", [NSLABS, 128, SLAB_CHUNKS * W_G], BF16, isOutput=False
    )
    out = nc.declare_dram_parameter(
        "out", [NSLABS, W_G, BINS_PER_SLAB * OUT_F], BF16, isOutput=True
    )

    groups_per_slab = SLAB_CHUNKS // (4 * CPB)  # 4 bins per PSUM bank -> 4 groups

    with tile.TileContext(nc) as tc:
        with (
            tc.tile_pool(name="t_pool", bufs=3) as t_pool,
            tc.tile_pool(name="c_pool", bufs=3) as c_pool,
            tc.tile_pool(name="o_pool", bufs=2) as o_pool,
            tc.tile_pool(name="ps_pool", bufs=4, space="PSUM") as ps_pool,
        ):
            for s in range(NSLABS):
                t_t = t_pool.tile([128, SLAB_CHUNKS * OUT_F], BF16)
                nc.sync.dma_start(t_t[:], tswz[s])
                c_t = c_pool.tile([128, SLAB_CHUNKS * W_G], BF16)
                nc.sync.dma_start(c_t[:], cswz[s])

                o_t = o_pool.tile([W_G, BINS_PER_SLAB * OUT_F], BF16)
                for g in range(groups_per_slab):
                    ps = ps_pool.tile([W_G, 4 * OUT_F], FP32, space="PSUM")
                    for j in range(4):  # bin within group
                        b = g * 4 + j
                        for k in range(CPB):
                            c = b * CPB + k
                            nc.tensor.matmul(
                                out=ps[:, OUT_F * j : OUT_F * (j + 1)],
                                lhsT=c_t[:, W_G * c : W_G * (c + 1)],
                                rhs=t_t[:, OUT_F * c : OUT_F * (c + 1)],
                                start=(k == 0),
                                stop=(k == CPB - 1),
                            )
                    nc.vector.tensor_copy(
                        o_t[:, 4 * OUT_F * g : 4 * OUT_F * (g + 1)], ps[:]
                    )
                nc.sync.dma_start(out[s], o_t[:])
    nc.compile()
    return nc


# ---------------- host-side packing ----------------
def _pack_core(rows_c, cols_c, vals_c, support_bf, bias_bf):
    """Pack one core's edges into (tswz, cswz, destmap).

    rows_c: local dest ids [0, 12500); cols_c: global src ids; vals_c: f32.
    Returns tswz [NSLABS,128,SLAB_CHUNKS*OUT_F] bf16,
            cswz [NSLABS,128,SLAB_CHUNKS*W_G] bf16,
            destmap [NBINS*W_G] int64 (-1 for unused slots).
    """
    deg = np.bincount(rows_c, minlength=D_PER_CORE)

    # balanced two-pointer binning with dest splitting: <=32 slots and
    # <=EDGE_CAP edges per bin. Take from the high-degree end when the
    # remaining capacity-per-slot exceeds the average degree, else from
    # the low end; a dest whose edges overflow the bin is split across
    # bins (host sums the partial outputs; bias counted once).
    order = np.argsort(-deg, kind="stable")
    degs = deg[order].astype(np.int64)
    n = len(order)
    avg = degs.sum() / D_PER_CORE
    piece_dest, piece_bin, piece_w, piece_take, piece_first = [], [], [], [], []
    i, j = 0, n - 1
    rem_front = int(degs[0])
    front_first = True
    b = 0

    def place(d, w, take, first):
        piece_dest.append(d)
        piece_bin.append(b)
        piece_w.append(w)
        piece_take.append(take)
        piece_first.append(first)

    while i <= j:
        slots, fill = 0, 0
        while slots < W_G and i <= j:
            cap = EDGE_CAP - fill
            if i == j:
                take = min(rem_front, cap)
                if take == 0 and rem_front > 0:
                    break
                place(int(order[i]), slots, take, front_first)
                front_first = False
                slots += 1
                fill += take
                rem_front -= take
                if rem_front == 0:
                    i += 1
                continue
            if (cap / (W_G - slots)) >= avg:
                take = min(rem_front, cap)
                if take < rem_front and take == 0:
                    break
                place(int(order[i]), slots, take, front_first)
                front_first = False
                slots += 1
                fill += take
                rem_front -= take
                if rem_front == 0:
                    i += 1
                    rem_front = int(degs[i]) if i < n else 0
                    front_first = True
            else:
                db = int(degs[j])
                if db <= cap:
                    place(int(order[j]), slots, db, True)
                    slots += 1
                    fill += db
                    j -= 1
                else:
                    if cap == 0:
                        break
                    take = min(rem_front, cap)
                    place(int(order[i]), slots, take, front_first)
                    front_first = False
                    slots += 1
                    fill += take
                    rem_front -= take
                    if rem_front == 0:
                        i += 1
                        rem_front = int(degs[i]) if i < n else 0
                        front_first = True
        b += 1
    nbins_used = b
    if nbins_used > NBINS:
        raise RuntimeError(f"bin overflow: {nbins_used} > {NBINS}")
    piece_dest = np.array(piece_dest, np.int64)
    piece_bin = np.array(piece_bin, np.int64)
    piece_w = np.array(piece_w, np.int64)
    piece_take = np.array(piece_take, np.int64)
    piece_first = np.array(piece_first, bool)

    destmap = np.full(NBINS * W_G, -1, np.int64)
    destmap[piece_bin * W_G + piece_w] = piece_dest
    bias_slot = np.zeros(NBINS * W_G, bool)
    bias_slot[(piece_bin * W_G + piece_w)[piece_first]] = True

    # per-edge piece: edges sorted by dest; rank within dest selects piece
    order_d = np.argsort(rows_c, kind="stable")
    dstart = np.zeros(D_PER_CORE + 1, np.int64)
    np.cumsum(deg, out=dstart[1:])
    rank = np.arange(len(rows_c)) - dstart[rows_c[order_d]]
    # piece boundaries per dest: order pieces by (dest, first-come)
    po = np.lexsort((np.arange(len(piece_dest)), piece_dest))
    p_d = piece_dest[po]
    p_take = piece_take[po]
    p_off = np.zeros(len(po), np.int64)
    newd = np.empty(len(po), bool)
    newd[0] = True
    np.not_equal(p_d[1:], p_d[:-1], out=newd[1:])
    csum = np.cumsum(p_take) - p_take
    base = np.where(newd, csum, 0)
    np.maximum.accumulate(base, out=base)
    p_off = csum - base  # start rank of each piece within its dest
    # map each edge (dest, rank) -> piece index via searchsorted per dest
    pstart_of_dest = np.zeros(D_PER_CORE + 1, np.int64)
    np.cumsum(np.bincount(p_d, minlength=D_PER_CORE), out=pstart_of_dest[1:])
    ed = rows_c[order_d]
    lo = pstart_of_dest[ed]
    hi = pstart_of_dest[ed + 1]
    # pieces per dest are tiny (1-2); resolve by comparing rank to offsets
    pidx = lo.copy()
    multi = hi - lo > 1
    if multi.any():
        # iterate piece levels (max pieces per dest is small)
        maxp = int((hi - lo).max())
        for lvl in range(1, maxp):
            cand = lo + lvl
            ok = (cand < hi) & (rank >= p_off[np.minimum(cand, len(p_off) - 1)])
            pidx = np.where(ok, cand, pidx)
    e_bin = np.empty(len(rows_c), np.int64)
    e_w = np.empty(len(rows_c), np.int64)
    e_bin[order_d] = piece_bin[po][pidx]
    e_w[order_d] = piece_w[po][pidx]

    # sort edges by (bin, src) and build per-bin unique source slots
    order_e = np.lexsort((cols_c, e_bin))
    eb = e_bin[order_e]
    ec = cols_c[order_e]
    ew = e_w[order_e]
    ev = vals_c[order_e]

    # unique (bin, src) pairs; slot = rank of pair within its bin
    key = eb * np.int64(N_NODES) + ec
    newpair = np.empty(len(key), bool)
    newpair[0] = True
    np.not_equal(key[1:], key[:-1], out=newpair[1:])
    pair_id = np.cumsum(newpair) - 1  # per-edge unique-pair index
    first_of_pair = np.flatnonzero(newpair)
    pair_bin = eb[first_of_pair]
    pair_src = ec[first_of_pair]
    # slot of pair within its bin
    bin_start_pair = np.zeros(nbins_used + 1, np.int64)
    np.cumsum(np.bincount(pair_bin, minlength=nbins_used), out=bin_start_pair[1:])
    pair_slot = np.arange(len(pair_bin)) - bin_start_pair[pair_bin]
    if pair_slot.max() >= CAP - 1:
        raise RuntimeError("unique-source overflow in a bin")
    e_slot = pair_slot[pair_id]  # per-edge table slot within bin

    # table row indices: [NBINS*CAP] -> source id (0 for padding)
    tidx = np.zeros(NBINS * CAP, np.int64)
    tidx[pair_bin * CAP + pair_slot] = pair_src
    t_all = support_bf[tidx]  # [NBINS*CAP, OUT_F] bf16
    # zero padding rows (slots with no pair) except bias row
    used = np.zeros(NBINS * CAP, bool)
    used[pair_bin * CAP + pair_slot] = True
    t_all[~used] = 0
    t_all[np.arange(NBINS) * CAP + (CAP - 1)] = bias_bf  # bias row

    # C matrix: [NBINS*CAP, W_G] fp32 accumulate then bf16
    c_all = np.zeros(NBINS * CAP * W_G, np.float32)
    np.add.at(c_all, (eb * CAP + e_slot) * W_G + ew, ev)
    c_all = c_all.reshape(NBINS * CAP, W_G)
    # bias row: 1 only on each dest's first slot (splits get bias once)
    bias_rows = np.arange(NBINS) * CAP + (CAP - 1)
    c_all[bias_rows] = bias_slot.reshape(NBINS, W_G).astype(np.float32)
    c_all = c_all.astype(BF16_NP)

    # swizzle: chunk rows -> partitions
    tswz = np.ascontiguousarray(
        t_all.reshape(NSLABS, SLAB_CHUNKS, 128, OUT_F)
        .transpose(0, 2, 1, 3)
        .reshape(NSLABS, 128, SLAB_CHUNKS * OUT_F)
    )
    cswz = np.ascontiguousarray(
        c_all.reshape(NSLABS, SLAB_CHUNKS, 128, W_G)
        .transpose(0, 2, 1, 3)
        .reshape(NSLABS, 128, SLAB_CHUNKS * W_G)
    )
    return tswz, cswz, destmap


def kernel(X_input, adj_row, adj_col, adj_val, W, bias):
    X_input = np.asarray(X_input, np.float32)
    adj_row = np.asarray(adj_row).astype(np.int64)
    adj_col = np.asarray(adj_col).astype(np.int64)
    adj_val = np.asarray(adj_val, np.float32)
    W = np.asarray(W, np.float32)
    bias = np.asarray(bias, np.float32)

    # ---- launch 1: support shards (bf16)
    nc1 = build_support_program()
    w_bf = np.ascontiguousarray(W.astype(BF16_NP).reshape(2, 128, OUT_F))
    in_maps1 = []
    for c in range(NCORES):
        sl = np.zeros((ROWS_PAD, IN_F), np.float32)
        lo = c * D_PER_CORE
        sl[:D_PER_CORE] = X_input[lo : lo + D_PER_CORE]
        xt = np.ascontiguousarray(
            sl.T.astype(BF16_NP).reshape(2, 128, ROWS_PAD)
        )
        in_maps1.append({"xt": xt, "w": w_bf})
    res1 = run_bass_kernel_spmd(nc1, in_maps1, list(range(NCORES)))
    kernel.last_res1 = res1
    shards = []
    for c in range(NCORES):
        s = res1.results[c]["sup"]  # [128, RTILES, OUT_F] bf16 (p-major)
        s = s.transpose(1, 0, 2).reshape(ROWS_PAD, OUT_F)[:D_PER_CORE]
        shards.append(s)
    support_bf = np.ascontiguousarray(np.concatenate(shards, axis=0)).astype(BF16_NP)

    # ---- host packing
    bias_bf = bias.astype(BF16_NP)
    core_of = adj_row // D_PER_CORE
    in_maps2 = []
    destmaps = []
    for c in range(NCORES):
        m = core_of == c
        tswz, cswz, destmap = _pack_core(
            adj_row[m] - c * D_PER_CORE,
            adj_col[m],
            adj_val[m],
            support_bf,
            bias_bf,
        )
        destmaps.append(destmap)
        in_maps2.append({"tswz": tswz, "cswz": cswz})

    # ---- launch 2
    nc2 = build_spmm_program()
    res2 = run_bass_kernel_spmd(nc2, in_maps2, list(range(NCORES)))
    kernel.last_res2 = res2
    out = np.empty((N_NODES, OUT_F), np.float32)
    for c in range(NCORES):
        o = res2.results[c]["out"]  # [NSLABS, W_G, BINS_PER_SLAB*OUT_F] bf16
        # slot (bin, w) -> o[s, w, bi*OUT_F : ...] where bin = s*BINS_PER_SLAB+bi
        o = (
            o.reshape(NSLABS, W_G, BINS_PER_SLAB, OUT_F)
            .transpose(0, 2, 1, 3)
            .reshape(NBINS * W_G, OUT_F)
        )
        dm = destmaps[c]
        valid = dm >= 0
        shard = np.zeros((D_PER_CORE, OUT_F), np.float32)
        np.add.at(shard, dm[valid], o[valid].astype(np.float32))
        out[c * D_PER_CORE : (c + 1) * D_PER_CORE] = shard
    return out


# revision 19
# speedup vs baseline: 15.1763x; 1.2982x over previous
"""GCN layer (X @ W, then COO spmm scatter-add by dest, + bias) on 8 trn2 cores.

Strategy (dest-sharded, per sharding hint):
  Launch 1 (SPMD): core c computes support shard = X[c*12500:(c+1)*12500] @ W
    in bf16 (fp32 PSUM accumulate). Host pre-transposes X so the contraction
    dim lands on partitions.
  Host: assembles full support (bf16); packs each core's 12500 dest nodes
    into bins of <=32 dests and <=511 edges (next-fit-decreasing by degree).
    Per bin: a halo table T of the bin's unique source support rows (<=512
    rows, last row = bias) and a values matrix C [512, 32] with
    C[src_slot, dest_slot] = edge val (bias row = 1). out_bin = C^T @ T.
  Launch 2 (SPMD): pure sequential streaming -- no gathers. Per 64-chunk
    slab: stream T [128, 64*128] and C [128, 64*32] (host pre-swizzled so
    chunk rows land on partitions), one matmul per chunk accumulating each
    bin's 4 chunks in PSUM ([32 dests, 128 feats] per bin, 4 bins per PSUM
    bank), DVE-evacuate to bf16, DMA out per slab. Host unpermutes dest
    rows and casts to fp32.
"""

import numpy as np
import ml_dtypes

import concourse.bass as bass  # noqa: F401  (kept for parity with tile API)
import concourse.tile as tile
from concourse import bacc, mybir
from concourse.bass_utils import run_bass_kernel_spmd

BF16_NP = ml_dtypes.bfloat16

# ---------------- problem constants (hardcoded; kernel.py is self-contained)
N_NODES = 100000
N_EDGES = 1600000
IN_F = 256
OUT_F = 128
NCORES = 8

D_PER_CORE = N_NODES // NCORES  # 12500 dest nodes per core

# launch-1 (support matmul) geometry
ROWS_PAD = 12544  # 98 * 128
RTILES = ROWS_PAD // 128

# launch-2 (streamed halo spmm) geometry
W_G = 32  # dests per bin
CAP = 512  # table rows per bin (4 chunks of 128; last row = bias)
EDGE_CAP = CAP - 1  # <=511 edges -> <=511 unique sources -> row 511 free
CPB = CAP // 128  # chunks per bin = 4
SLAB_CHUNKS = 128  # chunks per slab (32 bins)
BINS_PER_SLAB = SLAB_CHUNKS // CPB  # 32
NSLABS = 13
NBINS = NSLABS * BINS_PER_SLAB  # 416
NCHUNKS = NBINS * CPB  # 1728

FP32 = mybir.dt.float32
BF16 = mybir.dt.bfloat16


def _new_nc():
    return bacc.Bacc("TRN2", target_bir_lowering=False, debug=False)


# ---------------- launch 1: support^T = (X_shard @ W)^T (bf16) ----------------
# Weights stationary (lhsT = W chunk), X columns stream (N=448 per matmul).
L1_N = 448
L1_TILES = ROWS_PAD // L1_N  # 28
L1_GRP = 4  # psum tiles in flight per k-sweep


def build_support_program():
    nc = _new_nc()
    xt = nc.declare_dram_parameter("xt", [2, 128, ROWS_PAD], BF16, isOutput=False)
    w = nc.declare_dram_parameter("w", [2, 128, OUT_F], BF16, isOutput=False)
    sup = nc.declare_dram_parameter("sup", [128, ROWS_PAD], BF16, isOutput=True)

    with tile.TileContext(nc) as tc:
        with (
            tc.tile_pool(name="xt_pool", bufs=1) as xt_pool,
            tc.tile_pool(name="w_pool", bufs=1) as w_pool,
            tc.tile_pool(name="out_pool", bufs=1) as out_pool,
            tc.tile_pool(name="ps_pool", bufs=2, space="PSUM") as ps_pool,
        ):
            xt_t = xt_pool.tile([128, 2, ROWS_PAD], BF16)
            for k in range(2):
                nc.sync.dma_start(xt_t[:, k, :], xt[k])
            w_t = w_pool.tile([128, 2, OUT_F], BF16)
            for k in range(2):
                nc.sync.dma_start(w_t[:, k, :], w[k])

            sup_buf = out_pool.tile([128, ROWS_PAD], BF16)
            for g in range(L1_TILES // L1_GRP):
                pss = [
                    ps_pool.tile([128, L1_N], FP32, space="PSUM", name=f"ps{t}")
                    for t in range(L1_GRP)
                ]
                for k in range(2):
                    for t in range(L1_GRP):
                        i = g * L1_GRP + t
                        nc.tensor.matmul(
                            out=pss[t][:],
                            lhsT=w_t[:, k, :],
                            rhs=xt_t[:, k, L1_N * i : L1_N * (i + 1)],
                            start=(k == 0),
                            stop=(k == 1),
                        )
                for t in range(L1_GRP):
                    i = g * L1_GRP + t
                    nc.vector.tensor_copy(
                        sup_buf[:, L1_N * i : L1_N * (i + 1)], pss[t][:]
                    )
            nc.scalar.dma_start(sup[:], sup_buf[:])
    nc.compile()
    return nc


# ---------------- launch 2: streamed halo spmm ----------------
def build_spmm_program():
    nc = _new_nc()
    # per slab: T part [128, SLAB_CHUNKS*OUT_F] then C part [128, SLAB_CHUNKS*W_G]
    T_COLS = SLAB_CHUNKS * OUT_F
    stream = nc.declare_dram_parameter(
        "stream", [NSLABS, 128, SLAB_CHUNKS * (OUT_F + W_G)], BF16, isOutput=False
    )
    out = nc.declare_dram_parameter(
        "out", [NSLABS, W_G, BINS_PER_SLAB * OUT_F], BF16, isOutput=True
    )

    groups_per_slab = BINS_PER_SLAB // 4  # 4 bins per PSUM bank

    with tile.TileContext(nc) as tc:
        with (
            tc.tile_pool(name="s_pool", bufs=3) as s_pool,
            tc.tile_pool(name="o_pool", bufs=2) as o_pool,
            tc.tile_pool(name="ps_pool", bufs=4, space="PSUM") as ps_pool,
        ):
            for s in range(NSLABS):
                st = s_pool.tile([128, SLAB_CHUNKS * (OUT_F + W_G)], BF16)
                nc.sync.dma_start(st[:], stream[s])

                o_t = o_pool.tile([W_G, BINS_PER_SLAB * OUT_F], BF16)
                for g in range(groups_per_slab):
                    ps = ps_pool.tile([W_G, 4 * OUT_F], FP32, space="PSUM")
                    for j in range(4):  # bin within group
                        b = g * 4 + j
                        for k in range(CPB):
                            c = b * CPB + k
                            nc.tensor.matmul(
                                out=ps[:, OUT_F * j : OUT_F * (j + 1)],
                                lhsT=st[:, T_COLS + W_G * c : T_COLS + W_G * (c + 1)],
                                rhs=st[:, OUT_F * c : OUT_F * (c + 1)],
                                start=(k == 0),
                                stop=(k == CPB - 1),
                            )
                    nc.vector.tensor_copy(
                        o_t[:, 4 * OUT_F * g : 4 * OUT_F * (g + 1)], ps[:]
                    )
                nc.scalar.dma_start(out[s], o_t[:])
    nc.compile()
    return nc


# ---------------- host-side packing ----------------
def _pack_core(rows_c, cols_c, vals_c, support_bf, bias_bf):
    """Pack one core's edges into (tswz, cswz, destmap).

    rows_c: local dest ids [0, 12500); cols_c: global src ids; vals_c: f32.
    Returns stream [NSLABS,128,SLAB_CHUNKS*(OUT_F+W_G)] bf16 (T part then
    C part per slab) and destmap [NBINS*W_G] int64 (-1 for unused slots,
    multiple slots may map to one dest -- host sums).
    """
    deg = np.bincount(rows_c, minlength=D_PER_CORE)

    # balanced two-pointer binning with dest splitting: <=32 slots and
    # <=EDGE_CAP edges per bin. Take from the high-degree end when the
    # remaining capacity-per-slot exceeds the average degree, else from
    # the low end; a dest whose edges overflow the bin is split across
    # bins (host sums the partial outputs; bias counted once).
    order = np.argsort(-deg, kind="stable")
    degs = deg[order].astype(np.int64)
    n = len(order)
    avg = degs.sum() / D_PER_CORE
    piece_dest, piece_bin, piece_w, piece_take, piece_first = [], [], [], [], []
    i, j = 0, n - 1
    rem_front = int(degs[0])
    front_first = True
    b = 0

    def place(d, w, take, first):
        piece_dest.append(d)
        piece_bin.append(b)
        piece_w.append(w)
        piece_take.append(take)
        piece_first.append(first)

    while i <= j:
        slots, fill = 0, 0
        while slots < W_G and i <= j:
            cap = EDGE_CAP - fill
            if i == j:
                take = min(rem_front, cap)
                if take == 0 and rem_front > 0:
                    break
                place(int(order[i]), slots, take, front_first)
                front_first = False
                slots += 1
                fill += take
                rem_front -= take
                if rem_front == 0:
                    i += 1
                continue
            if (cap / (W_G - slots)) >= avg:
                take = min(rem_front, cap)
                if take < rem_front and take == 0:
                    break
                place(int(order[i]), slots, take, front_first)
                front_first = False
                slots += 1
                fill += take
                rem_front -= take
                if rem_front == 0:
                    i += 1
                    rem_front = int(degs[i]) if i < n else 0
                    front_first = True
            else:
                db = int(degs[j])
                if db <= cap:
                    place(int(order[j]), slots, db, True)
                    slots += 1
                    fill += db
                    j -= 1
                else:
                    if cap == 0:
                        break
                    take = min(rem_front, cap)
                    place(int(order[i]), slots, take, front_first)
                    front_first = False
                    slots += 1
                    fill += take
                    rem_front -= take
                    if rem_front == 0:
                        i += 1
                        rem_front = int(degs[i]) if i < n else 0
                        front_first = True
        b += 1
    nbins_used = b
    if nbins_used > NBINS:
        raise RuntimeError(f"bin overflow: {nbins_used} > {NBINS}")
    piece_dest = np.array(piece_dest, np.int64)
    piece_bin = np.array(piece_bin, np.int64)
    piece_w = np.array(piece_w, np.int64)
    piece_take = np.array(piece_take, np.int64)
    piece_first = np.array(piece_first, bool)

    destmap = np.full(NBINS * W_G, -1, np.int64)
    destmap[piece_bin * W_G + piece_w] = piece_dest
    bias_slot = np.zeros(NBINS * W_G, bool)
    bias_slot[(piece_bin * W_G + piece_w)[piece_first]] = True

    # per-edge piece: edges sorted by dest; rank within dest selects piece
    order_d = np.argsort(rows_c, kind="stable")
    dstart = np.zeros(D_PER_CORE + 1, np.int64)
    np.cumsum(deg, out=dstart[1:])
    rank = np.arange(len(rows_c)) - dstart[rows_c[order_d]]
    # piece boundaries per dest: order pieces by (dest, first-come)
    po = np.lexsort((np.arange(len(piece_dest)), piece_dest))
    p_d = piece_dest[po]
    p_take = piece_take[po]
    p_off = np.zeros(len(po), np.int64)
    newd = np.empty(len(po), bool)
    newd[0] = True
    np.not_equal(p_d[1:], p_d[:-1], out=newd[1:])
    csum = np.cumsum(p_take) - p_take
    base = np.where(newd, csum, 0)
    np.maximum.accumulate(base, out=base)
    p_off = csum - base  # start rank of each piece within its dest
    # map each edge (dest, rank) -> piece index via searchsorted per dest
    pstart_of_dest = np.zeros(D_PER_CORE + 1, np.int64)
    np.cumsum(np.bincount(p_d, minlength=D_PER_CORE), out=pstart_of_dest[1:])
    ed = rows_c[order_d]
    lo = pstart_of_dest[ed]
    hi = pstart_of_dest[ed + 1]
    # pieces per dest are tiny (1-2); resolve by comparing rank to offsets
    pidx = lo.copy()
    multi = hi - lo > 1
    if multi.any():
        # iterate piece levels (max pieces per dest is small)
        maxp = int((hi - lo).max())
        for lvl in range(1, maxp):
            cand = lo + lvl
            ok = (cand < hi) & (rank >= p_off[np.minimum(cand, len(p_off) - 1)])
            pidx = np.where(ok, cand, pidx)
    e_bin = np.empty(len(rows_c), np.int64)
    e_w = np.empty(len(rows_c), np.int64)
    e_bin[order_d] = piece_bin[po][pidx]
    e_w[order_d] = piece_w[po][pidx]

    # sort edges by (bin, src) and build per-bin unique source slots
    order_e = np.lexsort((cols_c, e_bin))
    eb = e_bin[order_e]
    ec = cols_c[order_e]
    ew = e_w[order_e]
    ev = vals_c[order_e]

    # unique (bin, src) pairs; slot = rank of pair within its bin
    key = eb * np.int64(N_NODES) + ec
    newpair = np.empty(len(key), bool)
    newpair[0] = True
    np.not_equal(key[1:], key[:-1], out=newpair[1:])
    pair_id = np.cumsum(newpair) - 1  # per-edge unique-pair index
    first_of_pair = np.flatnonzero(newpair)
    pair_bin = eb[first_of_pair]
    pair_src = ec[first_of_pair]
    # slot of pair within its bin
    bin_start_pair = np.zeros(nbins_used + 1, np.int64)
    np.cumsum(np.bincount(pair_bin, minlength=nbins_used), out=bin_start_pair[1:])
    pair_slot = np.arange(len(pair_bin)) - bin_start_pair[pair_bin]
    if pair_slot.max() >= CAP - 1:
        raise RuntimeError("unique-source overflow in a bin")
    e_slot = pair_slot[pair_id]  # per-edge table slot within bin

    # table row indices: [NBINS*CAP] -> source id (0 for padding)
    tidx = np.zeros(NBINS * CAP, np.int64)
    tidx[pair_bin * CAP + pair_slot] = pair_src
    t_all = support_bf[tidx]  # [NBINS*CAP, OUT_F] bf16
    # zero padding rows (slots with no pair) except bias row
    used = np.zeros(NBINS * CAP, bool)
    used[pair_bin * CAP + pair_slot] = True
    t_all[~used] = 0
    t_all[np.arange(NBINS) * CAP + (CAP - 1)] = bias_bf  # bias row

    # C matrix: [NBINS*CAP, W_G] fp32 accumulate then bf16
    c_all = np.zeros(NBINS * CAP * W_G, np.float32)
    np.add.at(c_all, (eb * CAP + e_slot) * W_G + ew, ev)
    c_all = c_all.reshape(NBINS * CAP, W_G)
    # bias row: 1 only on each dest's first slot (splits get bias once)
    bias_rows = np.arange(NBINS) * CAP + (CAP - 1)
    c_all[bias_rows] = bias_slot.reshape(NBINS, W_G).astype(np.float32)
    c_all = c_all.astype(BF16_NP)

    # swizzle: chunk rows -> partitions; merge T and C into one stream
    tswz = (
        t_all.reshape(NSLABS, SLAB_CHUNKS, 128, OUT_F)
        .transpose(0, 2, 1, 3)
        .reshape(NSLABS, 128, SLAB_CHUNKS * OUT_F)
    )
    cswz = (
        c_all.reshape(NSLABS, SLAB_CHUNKS, 128, W_G)
        .transpose(0, 2, 1, 3)
        .reshape(NSLABS, 128, SLAB_CHUNKS * W_G)
    )
    stream = np.ascontiguousarray(np.concatenate([tswz, cswz], axis=2))
    return stream, destmap


def kernel(X_input, adj_row, adj_col, adj_val, W, bias):
    X_input = np.asarray(X_input, np.float32)
    adj_row = np.asarray(adj_row).astype(np.int64)
    adj_col = np.asarray(adj_col).astype(np.int64)
    adj_val = np.asarray(adj_val, np.float32)
    W = np.asarray(W, np.float32)
    bias = np.asarray(bias, np.float32)

    # ---- launch 1: support shards (bf16)
    nc1 = build_support_program()
    w_bf = np.ascontiguousarray(W.astype(BF16_NP).reshape(2, 128, OUT_F))
    in_maps1 = []
    for c in range(NCORES):
        sl = np.zeros((ROWS_PAD, IN_F), np.float32)
        lo = c * D_PER_CORE
        sl[:D_PER_CORE] = X_input[lo : lo + D_PER_CORE]
        xt = np.ascontiguousarray(
            sl.T.astype(BF16_NP).reshape(2, 128, ROWS_PAD)
        )
        in_maps1.append({"xt": xt, "w": w_bf})
    res1 = run_bass_kernel_spmd(nc1, in_maps1, list(range(NCORES)))
    kernel.last_res1 = res1
    shards = []
    for c in range(NCORES):
        s = res1.results[c]["sup"]  # [128, ROWS_PAD] bf16 = support^T
        shards.append(s.T[:D_PER_CORE])
    support_bf = np.ascontiguousarray(np.concatenate(shards, axis=0)).astype(BF16_NP)

    # ---- host packing
    bias_bf = bias.astype(BF16_NP)
    core_of = adj_row // D_PER_CORE
    in_maps2 = []
    destmaps = []
    for c in range(NCORES):
        m = core_of == c
        stream, destmap = _pack_core(
            adj_row[m] - c * D_PER_CORE,
            adj_col[m],
            adj_val[m],
            support_bf,
            bias_bf,
        )
        destmaps.append(destmap)
        in_maps2.append({"stream": stream})

    # ---- launch 2
    nc2 = build_spmm_program()
    res2 = run_bass_kernel_spmd(nc2, in_maps2, list(range(NCORES)))
    kernel.last_res2 = res2
    out = np.empty((N_NODES, OUT_F), np.float32)
    for c in range(NCORES):
        o = res2.results[c]["out"]  # [NSLABS, W_G, BINS_PER_SLAB*OUT_F] bf16
        # slot (bin, w) -> o[s, w, bi*OUT_F : ...] where bin = s*BINS_PER_SLAB+bi
        o = (
            o.reshape(NSLABS, W_G, BINS_PER_SLAB, OUT_F)
            .transpose(0, 2, 1, 3)
            .reshape(NBINS * W_G, OUT_F)
        )
        dm = destmaps[c]
        valid = dm >= 0
        shard = np.zeros((D_PER_CORE, OUT_F), np.float32)
        np.add.at(shard, dm[valid], o[valid].astype(np.float32))
        out[c * D_PER_CORE : (c + 1) * D_PER_CORE] = shard
    return out


# revision 31
# speedup vs baseline: 15.4941x; 1.0209x over previous
"""GCN layer (X @ W, then COO spmm scatter-add by dest, + bias) on 8 trn2 cores.

Strategy (dest-sharded, per sharding hint):
  Launch 1 (SPMD): core c computes support shard = X[c*12500:(c+1)*12500] @ W
    in bf16 (fp32 PSUM accumulate). Host pre-transposes X so the contraction
    dim lands on partitions.
  Host: assembles full support (bf16); packs each core's 12500 dest nodes
    into bins of <=32 dests and <=511 edges (next-fit-decreasing by degree).
    Per bin: a halo table T of the bin's unique source support rows (<=512
    rows, last row = bias) and a values matrix C [512, 32] with
    C[src_slot, dest_slot] = edge val (bias row = 1). out_bin = C^T @ T.
  Launch 2 (SPMD): pure sequential streaming -- no gathers. Per 64-chunk
    slab: stream T [128, 64*128] and C [128, 64*32] (host pre-swizzled so
    chunk rows land on partitions), one matmul per chunk accumulating each
    bin's 4 chunks in PSUM ([32 dests, 128 feats] per bin, 4 bins per PSUM
    bank), DVE-evacuate to bf16, DMA out per slab. Host unpermutes dest
    rows and casts to fp32.
"""

import numpy as np
import ml_dtypes

import concourse.bass as bass  # noqa: F401  (kept for parity with tile API)
import concourse.tile as tile
from concourse import bacc, mybir
from concourse.bass_utils import run_bass_kernel_spmd

BF16_NP = ml_dtypes.bfloat16

# ---------------- problem constants (hardcoded; kernel.py is self-contained)
N_NODES = 100000
N_EDGES = 1600000
IN_F = 256
OUT_F = 128
NCORES = 8

D_PER_CORE = N_NODES // NCORES  # 12500 dest nodes per core

# launch-1 (support matmul) geometry
ROWS_PAD = 12544  # 98 * 128
RTILES = ROWS_PAD // 128

# launch-2 (streamed halo spmm) geometry
W_G = 32  # dests per bin
CAP = 512  # table rows per bin (4 chunks of 128), one row per edge
EDGE_CAP = CAP  # <=512 edges per bin
CPB = CAP // 128  # chunks per bin = 4
SLAB_CHUNKS = 128  # chunks per slab (32 bins)
BINS_PER_SLAB = SLAB_CHUNKS // CPB  # 32
NSLABS = 13
NBINS = NSLABS * BINS_PER_SLAB  # 416
NCHUNKS = NBINS * CPB  # 1728

FP32 = mybir.dt.float32
BF16 = mybir.dt.bfloat16


def _new_nc():
    return bacc.Bacc("TRN2", target_bir_lowering=False, debug=False)


# ---------------- launch 1: support^T = (X_shard @ W)^T (bf16) ----------------
# Weights stationary (lhsT = W chunk), X columns stream (N=448 per matmul).
L1_N = 448
L1_TILES = ROWS_PAD // L1_N  # 28
L1_GRP = 4  # psum tiles in flight per k-sweep


def build_support_program():
    nc = _new_nc()
    xt = nc.declare_dram_parameter("xt", [2, 128, ROWS_PAD], BF16, isOutput=False)
    w = nc.declare_dram_parameter("w", [2, 128, OUT_F], BF16, isOutput=False)
    sup = nc.declare_dram_parameter("sup", [128, ROWS_PAD], BF16, isOutput=True)

    with tile.TileContext(nc) as tc:
        with (
            tc.tile_pool(name="xt_pool", bufs=1) as xt_pool,
            tc.tile_pool(name="w_pool", bufs=1) as w_pool,
            tc.tile_pool(name="out_pool", bufs=1) as out_pool,
            tc.tile_pool(name="ps_pool", bufs=2, space="PSUM") as ps_pool,
        ):
            xt_t = xt_pool.tile([128, 2, ROWS_PAD], BF16)
            half = ROWS_PAD // 2
            for h in range(2):
                for k in range(2):
                    nc.sync.dma_start(
                        xt_t[:, k, half * h : half * (h + 1)],
                        xt[k, :, half * h : half * (h + 1)],
                    )
            w_t = w_pool.tile([128, 2, OUT_F], BF16)
            for k in range(2):
                nc.sync.dma_start(w_t[:, k, :], w[k])

            sup_buf = out_pool.tile([128, ROWS_PAD], BF16)
            ngrp = L1_TILES // L1_GRP
            gcols = L1_N * L1_GRP
            for g in range(ngrp):
                pss = [
                    ps_pool.tile([128, L1_N], FP32, space="PSUM", name=f"ps{t}")
                    for t in range(L1_GRP)
                ]
                for k in range(2):
                    for t in range(L1_GRP):
                        i = g * L1_GRP + t
                        nc.tensor.matmul(
                            out=pss[t][:],
                            lhsT=w_t[:, k, :],
                            rhs=xt_t[:, k, L1_N * i : L1_N * (i + 1)],
                            start=(k == 0),
                            stop=(k == 1),
                        )
                for t in range(L1_GRP):
                    i = g * L1_GRP + t
                    nc.vector.tensor_copy(
                        sup_buf[:, L1_N * i : L1_N * (i + 1)], pss[t][:]
                    )
                nc.scalar.dma_start(
                    sup[:, gcols * g : gcols * (g + 1)],
                    sup_buf[:, gcols * g : gcols * (g + 1)],
                )
    nc.compile()
    return nc


# ---------------- launch 2: streamed halo spmm ----------------
def build_spmm_program():
    nc = _new_nc()
    # per slab: [w: SLAB_CHUNKS cols][v: SLAB_CHUNKS cols][T: SLAB_CHUNKS*OUT_F]
    VOFF = SLAB_CHUNKS
    TOFF = 2 * SLAB_CHUNKS
    SCOLS = SLAB_CHUNKS * (OUT_F + 2)
    stream = nc.declare_dram_parameter(
        "stream", [NSLABS, 128, SCOLS], BF16, isOutput=False
    )
    iota = nc.declare_dram_parameter("iota", [128, 1, W_G], BF16, isOutput=False)
    bias_rep = nc.declare_dram_parameter(
        "bias_rep", [W_G, 4 * OUT_F], BF16, isOutput=False
    )
    out = nc.declare_dram_parameter(
        "out", [NSLABS, W_G, BINS_PER_SLAB * OUT_F], BF16, isOutput=True
    )

    groups_per_slab = BINS_PER_SLAB // 4  # 4 bins per PSUM bank
    NQ = 4  # quarter loads per slab

    with tile.TileContext(nc) as tc:
        with (
            tc.tile_pool(name="const_pool", bufs=1) as const_pool,
            tc.tile_pool(name="s_pool", bufs=3) as s_pool,
            tc.tile_pool(name="c_pool", bufs=2) as c_pool,
            tc.tile_pool(name="o_pool", bufs=2) as o_pool,
            tc.tile_pool(name="ps_pool", bufs=4, space="PSUM") as ps_pool,
        ):
            iota_t = const_pool.tile([128, 1, W_G], BF16)
            nc.sync.dma_start(iota_t[:], iota[:])
            bias_t = const_pool.tile([W_G, 4 * OUT_F], BF16)
            nc.sync.dma_start(bias_t[:], bias_rep[:])

            for s in range(NSLABS):
                st = s_pool.tile([128, SCOLS], BF16)
                q = SCOLS // NQ
                for qi in range(NQ):
                    nc.sync.dma_start(
                        st[:, q * qi : q * (qi + 1)],
                        stream[s, :, q * qi : q * (qi + 1)],
                    )

                # build C on-chip: cb = (w == iota) * v
                ceq = c_pool.tile([128, SLAB_CHUNKS, W_G], BF16)
                nc.vector.tensor_tensor(
                    out=ceq[:],
                    in0=st[:, 0:SLAB_CHUNKS].to_broadcast([128, SLAB_CHUNKS, W_G]),
                    in1=iota_t[:].to_broadcast([128, SLAB_CHUNKS, W_G]),
                    op=mybir.AluOpType.is_equal,
                )
                cb = c_pool.tile([128, SLAB_CHUNKS, W_G], BF16)
                nc.vector.tensor_tensor(
                    out=cb[:],
                    in0=ceq[:],
                    in1=st[:, VOFF : VOFF + SLAB_CHUNKS].to_broadcast(
                        [128, SLAB_CHUNKS, W_G]
                    ),
                    op=mybir.AluOpType.mult,
                )

                o_t = o_pool.tile([W_G, BINS_PER_SLAB * OUT_F], BF16)
                for g in range(groups_per_slab):
                    ps = ps_pool.tile([W_G, 4 * OUT_F], FP32, space="PSUM")
                    for j in range(4):  # bin within group
                        b = g * 4 + j
                        for k in range(CPB):
                            c = b * CPB + k
                            nc.tensor.matmul(
                                out=ps[:, OUT_F * j : OUT_F * (j + 1)],
                                lhsT=cb[:, c, :],
                                rhs=st[:, TOFF + OUT_F * c : TOFF + OUT_F * (c + 1)],
                                start=(k == 0),
                                stop=(k == CPB - 1),
                            )
                    nc.vector.tensor_tensor(
                        out=o_t[:, 4 * OUT_F * g : 4 * OUT_F * (g + 1)],
                        in0=ps[:],
                        in1=bias_t[:],
                        op=mybir.AluOpType.add,
                    )
                nc.scalar.dma_start(out[s], o_t[:])
    nc.compile()
    return nc


# ---------------- host-side packing ----------------
def _pack_core(rows_c, cols_c, vals_c, support_bf):
    """Pack one core's edges into (tswz, cswz, destmap).

    rows_c: local dest ids [0, 12500); cols_c: global src ids; vals_c: f32.
    Returns stream [NSLABS,128,SLAB_CHUNKS*(OUT_F+2)] bf16 ([w|v|T] per
    slab) and destmap [NBINS*W_G] int64 (-1 for unused slots, multiple
    slots may map to one dest -- host sums, then de-dupes bias).
    """
    deg = np.bincount(rows_c, minlength=D_PER_CORE)

    # balanced two-pointer binning with dest splitting: <=32 slots and
    # <=EDGE_CAP edges per bin. Take from the high-degree end when the
    # remaining capacity-per-slot exceeds the average degree, else from
    # the low end; a dest whose edges overflow the bin is split across
    # bins (host sums the partial outputs; bias counted once).
    order = np.argsort(-deg, kind="stable")
    degs = deg[order].astype(np.int64)
    n = len(order)
    avg = degs.sum() / D_PER_CORE
    piece_dest, piece_bin, piece_w, piece_take, piece_first = [], [], [], [], []
    i, j = 0, n - 1
    rem_front = int(degs[0])
    front_first = True
    b = 0

    def place(d, w, take, first):
        piece_dest.append(d)
        piece_bin.append(b)
        piece_w.append(w)
        piece_take.append(take)
        piece_first.append(first)

    while i <= j:
        slots, fill = 0, 0
        while slots < W_G and i <= j:
            cap = EDGE_CAP - fill
            if i == j:
                take = min(rem_front, cap)
                if take == 0 and rem_front > 0:
                    break
                place(int(order[i]), slots, take, front_first)
                front_first = False
                slots += 1
                fill += take
                rem_front -= take
                if rem_front == 0:
                    i += 1
                continue
            if (cap / (W_G - slots)) >= avg:
                take = min(rem_front, cap)
                if take < rem_front and take == 0:
                    break
                place(int(order[i]), slots, take, front_first)
                front_first = False
                slots += 1
                fill += take
                rem_front -= take
                if rem_front == 0:
                    i += 1
                    rem_front = int(degs[i]) if i < n else 0
                    front_first = True
            else:
                db = int(degs[j])
                if db <= cap:
                    place(int(order[j]), slots, db, True)
                    slots += 1
                    fill += db
                    j -= 1
                else:
                    if cap == 0:
                        break
                    take = min(rem_front, cap)
                    place(int(order[i]), slots, take, front_first)
                    front_first = False
                    slots += 1
                    fill += take
                    rem_front -= take
                    if rem_front == 0:
                        i += 1
                        rem_front = int(degs[i]) if i < n else 0
                        front_first = True
        b += 1
    nbins_used = b
    if nbins_used > NBINS:
        raise RuntimeError(f"bin overflow: {nbins_used} > {NBINS}")
    piece_dest = np.array(piece_dest, np.int64)
    piece_bin = np.array(piece_bin, np.int64)
    piece_w = np.array(piece_w, np.int64)
    piece_take = np.array(piece_take, np.int64)
    piece_first = np.array(piece_first, bool)

    destmap = np.full(NBINS * W_G, -1, np.int64)
    destmap[piece_bin * W_G + piece_w] = piece_dest

    # per-edge piece: edges sorted by dest; rank within dest selects piece
    order_d = np.argsort(rows_c, kind="stable")
    dstart = np.zeros(D_PER_CORE + 1, np.int64)
    np.cumsum(deg, out=dstart[1:])
    rank = np.arange(len(rows_c)) - dstart[rows_c[order_d]]
    # piece boundaries per dest: order pieces by (dest, first-come)
    po = np.lexsort((np.arange(len(piece_dest)), piece_dest))
    p_d = piece_dest[po]
    p_take = piece_take[po]
    p_off = np.zeros(len(po), np.int64)
    newd = np.empty(len(po), bool)
    newd[0] = True
    np.not_equal(p_d[1:], p_d[:-1], out=newd[1:])
    csum = np.cumsum(p_take) - p_take
    base = np.where(newd, csum, 0)
    np.maximum.accumulate(base, out=base)
    p_off = csum - base  # start rank of each piece within its dest
    # map each edge (dest, rank) -> piece index via searchsorted per dest
    pstart_of_dest = np.zeros(D_PER_CORE + 1, np.int64)
    np.cumsum(np.bincount(p_d, minlength=D_PER_CORE), out=pstart_of_dest[1:])
    ed = rows_c[order_d]
    lo = pstart_of_dest[ed]
    hi = pstart_of_dest[ed + 1]
    # pieces per dest are tiny (1-2); resolve by comparing rank to offsets
    pidx = lo.copy()
    multi = hi - lo > 1
    if multi.any():
        # iterate piece levels (max pieces per dest is small)
        maxp = int((hi - lo).max())
        for lvl in range(1, maxp):
            cand = lo + lvl
            ok = (cand < hi) & (rank >= p_off[np.minimum(cand, len(p_off) - 1)])
            pidx = np.where(ok, cand, pidx)
    e_bin = np.empty(len(rows_c), np.int64)
    e_w = np.empty(len(rows_c), np.int64)
    e_bin[order_d] = piece_bin[po][pidx]
    e_w[order_d] = piece_w[po][pidx]

    # one table slot per edge: sort edges by bin, slot = rank within bin
    order_e = np.argsort(e_bin, kind="stable")
    eb = e_bin[order_e]
    ec = cols_c[order_e]
    ew = e_w[order_e]
    ev = vals_c[order_e]
    bin_start = np.zeros(nbins_used + 1, np.int64)
    np.cumsum(np.bincount(eb, minlength=nbins_used), out=bin_start[1:])
    e_slot = np.arange(len(eb)) - bin_start[eb]
    if len(e_slot) and e_slot.max() >= CAP:
        raise RuntimeError("edge overflow in a bin")

    rows_idx = eb * CAP + e_slot
    tidx = np.zeros(NBINS * CAP, np.int64)
    tidx[rows_idx] = ec
    t_all = support_bf[tidx]  # [NBINS*CAP, OUT_F] bf16 (pad rows killed by v=0)
    w_all = np.zeros(NBINS * CAP, np.float32)
    w_all[rows_idx] = ew
    v_all = np.zeros(NBINS * CAP, np.float32)
    v_all[rows_idx] = ev

    # swizzle: chunk rows -> partitions; stream = [w | v | T] per slab
    wswz = w_all.astype(BF16_NP).reshape(NSLABS, SLAB_CHUNKS, 128).transpose(0, 2, 1)
    vswz = v_all.astype(BF16_NP).reshape(NSLABS, SLAB_CHUNKS, 128).transpose(0, 2, 1)
    tswz = (
        t_all.reshape(NSLABS, SLAB_CHUNKS, 128, OUT_F)
        .transpose(0, 2, 1, 3)
        .reshape(NSLABS, 128, SLAB_CHUNKS * OUT_F)
    )
    stream = np.ascontiguousarray(np.concatenate([wswz, vswz, tswz], axis=2))
    return stream, destmap


def kernel(X_input, adj_row, adj_col, adj_val, W, bias):
    X_input = np.asarray(X_input, np.float32)
    adj_row = np.asarray(adj_row).astype(np.int64)
    adj_col = np.asarray(adj_col).astype(np.int64)
    adj_val = np.asarray(adj_val, np.float32)
    W = np.asarray(W, np.float32)
    bias = np.asarray(bias, np.float32)

    # ---- launch 1: support shards (bf16)
    nc1 = build_support_program()
    w_bf = np.ascontiguousarray(W.astype(BF16_NP).reshape(2, 128, OUT_F))
    in_maps1 = []
    for c in range(NCORES):
        sl = np.zeros((ROWS_PAD, IN_F), np.float32)
        lo = c * D_PER_CORE
        sl[:D_PER_CORE] = X_input[lo : lo + D_PER_CORE]
        xt = np.ascontiguousarray(
            sl.T.astype(BF16_NP).reshape(2, 128, ROWS_PAD)
        )
        in_maps1.append({"xt": xt, "w": w_bf})
    res1 = run_bass_kernel_spmd(nc1, in_maps1, list(range(NCORES)))
    kernel.last_res1 = res1
    shards = []
    for c in range(NCORES):
        s = res1.results[c]["sup"]  # [128, ROWS_PAD] bf16 = support^T
        shards.append(s.T[:D_PER_CORE])
    support_bf = np.ascontiguousarray(np.concatenate(shards, axis=0)).astype(BF16_NP)

    # ---- host packing
    bias_bf = bias.astype(BF16_NP)
    iota_arr = np.ascontiguousarray(
        np.broadcast_to(np.arange(W_G, dtype=np.float32), (128, 1, W_G))
    ).astype(BF16_NP)
    bias_rep = np.ascontiguousarray(np.tile(bias_bf, (W_G, 4)))
    core_of = adj_row // D_PER_CORE
    in_maps2 = []
    destmaps = []
    for c in range(NCORES):
        m = core_of == c
        stream, destmap = _pack_core(
            adj_row[m] - c * D_PER_CORE,
            adj_col[m],
            adj_val[m],
            support_bf,
        )
        destmaps.append(destmap)
        in_maps2.append({"stream": stream, "iota": iota_arr, "bias_rep": bias_rep})

    # ---- launch 2
    nc2 = build_spmm_program()
    res2 = run_bass_kernel_spmd(nc2, in_maps2, list(range(NCORES)))
    kernel.last_res2 = res2
    out = np.empty((N_NODES, OUT_F), np.float32)
    for c in range(NCORES):
        o = res2.results[c]["out"]  # [NSLABS, W_G, BINS_PER_SLAB*OUT_F] bf16
        # slot (bin, w) -> o[s, w, bi*OUT_F : ...] where bin = s*BINS_PER_SLAB+bi
        o = (
            o.reshape(NSLABS, W_G, BINS_PER_SLAB, OUT_F)
            .transpose(0, 2, 1, 3)
            .reshape(NBINS * W_G, OUT_F)
        )
        dm = destmaps[c]
        valid = dm >= 0
        shard = np.zeros((D_PER_CORE, OUT_F), np.float32)
        np.add.at(shard, dm[valid], o[valid].astype(np.float32))
        # every slot's evac added bias once; keep exactly one per dest
        counts = np.bincount(dm[valid], minlength=D_PER_CORE)
        shard -= (counts - 1)[:, None] * bias_bf.astype(np.float32)
        out[c * D_PER_CORE : (c + 1) * D_PER_CORE] = shard
    return out


# revision 38
# speedup vs baseline: 17.4474x; 1.1261x over previous
"""GCN layer (X @ W, then COO spmm scatter-add by dest, + bias) on 8 trn2 cores.

Strategy (dest-sharded, per sharding hint):
  Launch 1 (SPMD): core c computes support shard = X[c*12500:(c+1)*12500] @ W
    in bf16 (fp32 PSUM accumulate). Host pre-transposes X so the contraction
    dim lands on partitions.
  Host: assembles full support (bf16); packs each core's 12500 dest nodes
    into bins of <=32 dests and <=511 edges (next-fit-decreasing by degree).
    Per bin: a halo table T of the bin's unique source support rows (<=512
    rows, last row = bias) and a values matrix C [512, 32] with
    C[src_slot, dest_slot] = edge val (bias row = 1). out_bin = C^T @ T.
  Launch 2 (SPMD): pure sequential streaming -- no gathers. Per 64-chunk
    slab: stream T [128, 64*128] and C [128, 64*32] (host pre-swizzled so
    chunk rows land on partitions), one matmul per chunk accumulating each
    bin's 4 chunks in PSUM ([32 dests, 128 feats] per bin, 4 bins per PSUM
    bank), DVE-evacuate to bf16, DMA out per slab. Host unpermutes dest
    rows and casts to fp32.
"""

import numpy as np
import ml_dtypes

import concourse.bass as bass  # noqa: F401  (kept for parity with tile API)
import concourse.tile as tile
from concourse import bacc, mybir
from concourse.bass_utils import run_bass_kernel_spmd

BF16_NP = ml_dtypes.bfloat16

# ---------------- problem constants (hardcoded; kernel.py is self-contained)
N_NODES = 100000
N_EDGES = 1600000
IN_F = 256
OUT_F = 128
NCORES = 8

D_PER_CORE = N_NODES // NCORES  # 12500 dest nodes per core

# launch-1 (support matmul) geometry
ROWS_PAD = 12544  # 98 * 128
RTILES = ROWS_PAD // 128

# launch-2 (streamed halo spmm) geometry
W_G = 32  # dests per bin
CAP = 512  # table rows per bin (4 chunks of 128), one row per edge
EDGE_CAP = CAP  # <=512 edges per bin
CPB = CAP // 128  # chunks per bin = 4
SLAB_CHUNKS = 128  # chunks per slab (32 bins)
BINS_PER_SLAB = SLAB_CHUNKS // CPB  # 32
NSLABS = 13
NBINS = NSLABS * BINS_PER_SLAB  # 416
NCHUNKS = NBINS * CPB  # 1728

FP32 = mybir.dt.float32
BF16 = mybir.dt.bfloat16


def _new_nc():
    return bacc.Bacc("TRN2", target_bir_lowering=False, debug=False)


# ---------------- launch 1: support^T = (X_shard @ W)^T (bf16) ----------------
# Weights stationary (lhsT = W chunk), X columns stream (N=448 per matmul).
L1_N = 448
L1_TILES = ROWS_PAD // L1_N  # 28
L1_GRP = 4  # psum tiles in flight per k-sweep


def build_support_program():
    nc = _new_nc()
    xt = nc.declare_dram_parameter("xt", [2, 128, ROWS_PAD], BF16, isOutput=False)
    w = nc.declare_dram_parameter("w", [2, 128, OUT_F], BF16, isOutput=False)
    sup = nc.declare_dram_parameter("sup", [128, ROWS_PAD], BF16, isOutput=True)

    with tile.TileContext(nc) as tc:
        with (
            tc.tile_pool(name="xt_pool", bufs=1) as xt_pool,
            tc.tile_pool(name="w_pool", bufs=1) as w_pool,
            tc.tile_pool(name="out_pool", bufs=1) as out_pool,
            tc.tile_pool(name="ps_pool", bufs=2, space="PSUM") as ps_pool,
        ):
            w_t = w_pool.tile([128, 2, OUT_F], BF16)
            for k in range(2):
                nc.sync.dma_start(w_t[:, k, :], w[k])
            xt_t = xt_pool.tile([128, 2, ROWS_PAD], BF16)
            half = ROWS_PAD // 2
            for h in range(2):
                for k in range(2):
                    nc.sync.dma_start(
                        xt_t[:, k, half * h : half * (h + 1)],
                        xt[k, :, half * h : half * (h + 1)],
                    )

            sup_buf = out_pool.tile([128, ROWS_PAD], BF16)
            ngrp = L1_TILES // L1_GRP
            gcols = L1_N * L1_GRP
            for g in range(ngrp):
                pss = [
                    ps_pool.tile([128, L1_N], FP32, space="PSUM", name=f"ps{t}")
                    for t in range(L1_GRP)
                ]
                for k in range(2):
                    for t in range(L1_GRP):
                        i = g * L1_GRP + t
                        nc.tensor.matmul(
                            out=pss[t][:],
                            lhsT=w_t[:, k, :],
                            rhs=xt_t[:, k, L1_N * i : L1_N * (i + 1)],
                            start=(k == 0),
                            stop=(k == 1),
                        )
                for t in range(L1_GRP):
                    i = g * L1_GRP + t
                    nc.vector.tensor_copy(
                        sup_buf[:, L1_N * i : L1_N * (i + 1)], pss[t][:]
                    )
                nc.scalar.dma_start(
                    sup[:, gcols * g : gcols * (g + 1)],
                    sup_buf[:, gcols * g : gcols * (g + 1)],
                )
    nc.compile()
    return nc


# ---------------- launch 2: streamed halo spmm ----------------
def build_spmm_program():
    nc = _new_nc()
    # per slab: [w: SLAB_CHUNKS cols][v: SLAB_CHUNKS cols][T: SLAB_CHUNKS*OUT_F]
    VOFF = SLAB_CHUNKS
    TOFF = 2 * SLAB_CHUNKS
    SCOLS = SLAB_CHUNKS * (OUT_F + 2)
    stream = nc.declare_dram_parameter(
        "stream", [NSLABS, 128, SCOLS], BF16, isOutput=False
    )
    iota = nc.declare_dram_parameter("iota", [128, 1, W_G], BF16, isOutput=False)
    out = nc.declare_dram_parameter(
        "out", [NSLABS, W_G, BINS_PER_SLAB * OUT_F], BF16, isOutput=True
    )

    groups_per_slab = BINS_PER_SLAB // 4  # 4 bins per PSUM bank
    NQ = 4  # quarter loads per slab

    with tile.TileContext(nc) as tc:
        with (
            tc.tile_pool(name="const_pool", bufs=1) as const_pool,
            tc.tile_pool(name="s_pool", bufs=3) as s_pool,
            tc.tile_pool(name="c_pool", bufs=2) as c_pool,
            tc.tile_pool(name="o_pool", bufs=2) as o_pool,
            tc.tile_pool(name="ps_pool", bufs=4, space="PSUM") as ps_pool,
        ):
            iota_t = const_pool.tile([128, 1, W_G], BF16)
            nc.sync.dma_start(iota_t[:], iota[:])

            for s in range(NSLABS):
                st = s_pool.tile([128, SCOLS], BF16)
                q = SCOLS // NQ
                for qi in range(NQ):
                    nc.sync.dma_start(
                        st[:, q * qi : q * (qi + 1)],
                        stream[s, :, q * qi : q * (qi + 1)],
                    )

                # build C on-chip: cb = (w == iota) * v
                ceq = c_pool.tile([128, SLAB_CHUNKS, W_G], BF16)
                nc.vector.tensor_tensor(
                    out=ceq[:],
                    in0=st[:, 0:SLAB_CHUNKS].to_broadcast([128, SLAB_CHUNKS, W_G]),
                    in1=iota_t[:].to_broadcast([128, SLAB_CHUNKS, W_G]),
                    op=mybir.AluOpType.is_equal,
                )
                cb = c_pool.tile([128, SLAB_CHUNKS, W_G], BF16)
                nc.vector.tensor_tensor(
                    out=cb[:],
                    in0=ceq[:],
                    in1=st[:, VOFF : VOFF + SLAB_CHUNKS].to_broadcast(
                        [128, SLAB_CHUNKS, W_G]
                    ),
                    op=mybir.AluOpType.mult,
                )

                o_t = o_pool.tile([W_G, BINS_PER_SLAB * OUT_F], BF16)
                for g in range(groups_per_slab):
                    ps = ps_pool.tile([W_G, 4 * OUT_F], FP32, space="PSUM")
                    for j in range(4):  # bin within group
                        b = g * 4 + j
                        for k in range(CPB):
                            c = b * CPB + k
                            nc.tensor.matmul(
                                out=ps[:, OUT_F * j : OUT_F * (j + 1)],
                                lhsT=cb[:, c, :],
                                rhs=st[:, TOFF + OUT_F * c : TOFF + OUT_F * (c + 1)],
                                start=(k == 0),
                                stop=(k == CPB - 1),
                            )
                    nc.scalar.copy(
                        out=o_t[:, 4 * OUT_F * g : 4 * OUT_F * (g + 1)],
                        in_=ps[:],
                    )
                nc.scalar.dma_start(out[s], o_t[:])
    nc.compile()
    return nc


# ---------------- host-side packing ----------------
def _pack_core(rows_c, cols_c, vals_c, support_bf):
    """Pack one core's edges into (tswz, cswz, destmap).

    rows_c: local dest ids [0, 12500); cols_c: global src ids; vals_c: f32.
    Returns stream [NSLABS,128,SLAB_CHUNKS*(OUT_F+2)] bf16 ([w|v|T] per
    slab) and destmap [NBINS*W_G] int64 (-1 for unused slots, multiple
    slots may map to one dest -- host sums, then de-dupes bias).
    """
    deg = np.bincount(rows_c, minlength=D_PER_CORE)

    # balanced two-pointer binning with dest splitting: <=32 slots and
    # <=EDGE_CAP edges per bin. Take from the high-degree end when the
    # remaining capacity-per-slot exceeds the average degree, else from
    # the low end; a dest whose edges overflow the bin is split across
    # bins (host sums the partial outputs; bias counted once).
    order = np.argsort(-deg, kind="stable")
    degs = deg[order].astype(np.int64)
    n = len(order)
    avg = degs.sum() / D_PER_CORE
    piece_dest, piece_bin, piece_w, piece_take, piece_first = [], [], [], [], []
    i, j = 0, n - 1
    rem_front = int(degs[0])
    front_first = True
    b = 0

    def place(d, w, take, first):
        piece_dest.append(d)
        piece_bin.append(b)
        piece_w.append(w)
        piece_take.append(take)
        piece_first.append(first)

    while i <= j:
        slots, fill = 0, 0
        while slots < W_G and i <= j:
            cap = EDGE_CAP - fill
            if i == j:
                take = min(rem_front, cap)
                if take == 0 and rem_front > 0:
                    break
                place(int(order[i]), slots, take, front_first)
                front_first = False
                slots += 1
                fill += take
                rem_front -= take
                if rem_front == 0:
                    i += 1
                continue
            if (cap / (W_G - slots)) >= avg:
                take = min(rem_front, cap)
                if take < rem_front and take == 0:
                    break
                place(int(order[i]), slots, take, front_first)
                front_first = False
                slots += 1
                fill += take
                rem_front -= take
                if rem_front == 0:
                    i += 1
                    rem_front = int(degs[i]) if i < n else 0
                    front_first = True
            else:
                db = int(degs[j])
                if db <= cap:
                    place(int(order[j]), slots, db, True)
                    slots += 1
                    fill += db
                    j -= 1
                else:
                    if cap == 0:
                        break
                    take = min(rem_front, cap)
                    place(int(order[i]), slots, take, front_first)
                    front_first = False
                    slots += 1
                    fill += take
                    rem_front -= take
                    if rem_front == 0:
                        i += 1
                        rem_front = int(degs[i]) if i < n else 0
                        front_first = True
        b += 1
    nbins_used = b
    if nbins_used > NBINS:
        raise RuntimeError(f"bin overflow: {nbins_used} > {NBINS}")
    piece_dest = np.array(piece_dest, np.int64)
    piece_bin = np.array(piece_bin, np.int64)
    piece_w = np.array(piece_w, np.int64)
    piece_take = np.array(piece_take, np.int64)
    piece_first = np.array(piece_first, bool)

    destmap = np.full(NBINS * W_G, -1, np.int64)
    destmap[piece_bin * W_G + piece_w] = piece_dest

    # per-edge piece: edges sorted by dest; rank within dest selects piece
    order_d = np.argsort(rows_c, kind="stable")
    dstart = np.zeros(D_PER_CORE + 1, np.int64)
    np.cumsum(deg, out=dstart[1:])
    rank = np.arange(len(rows_c)) - dstart[rows_c[order_d]]
    # piece boundaries per dest: order pieces by (dest, first-come)
    po = np.lexsort((np.arange(len(piece_dest)), piece_dest))
    p_d = piece_dest[po]
    p_take = piece_take[po]
    p_off = np.zeros(len(po), np.int64)
    newd = np.empty(len(po), bool)
    newd[0] = True
    np.not_equal(p_d[1:], p_d[:-1], out=newd[1:])
    csum = np.cumsum(p_take) - p_take
    base = np.where(newd, csum, 0)
    np.maximum.accumulate(base, out=base)
    p_off = csum - base  # start rank of each piece within its dest
    # map each edge (dest, rank) -> piece index via searchsorted per dest
    pstart_of_dest = np.zeros(D_PER_CORE + 1, np.int64)
    np.cumsum(np.bincount(p_d, minlength=D_PER_CORE), out=pstart_of_dest[1:])
    ed = rows_c[order_d]
    lo = pstart_of_dest[ed]
    hi = pstart_of_dest[ed + 1]
    # pieces per dest are tiny (1-2); resolve by comparing rank to offsets
    pidx = lo.copy()
    multi = hi - lo > 1
    if multi.any():
        # iterate piece levels (max pieces per dest is small)
        maxp = int((hi - lo).max())
        for lvl in range(1, maxp):
            cand = lo + lvl
            ok = (cand < hi) & (rank >= p_off[np.minimum(cand, len(p_off) - 1)])
            pidx = np.where(ok, cand, pidx)
    e_bin = np.empty(len(rows_c), np.int64)
    e_w = np.empty(len(rows_c), np.int64)
    e_bin[order_d] = piece_bin[po][pidx]
    e_w[order_d] = piece_w[po][pidx]

    # one table slot per edge: sort edges by bin, slot = rank within bin
    order_e = np.argsort(e_bin, kind="stable")
    eb = e_bin[order_e]
    ec = cols_c[order_e]
    ew = e_w[order_e]
    ev = vals_c[order_e]
    bin_start = np.zeros(nbins_used + 1, np.int64)
    np.cumsum(np.bincount(eb, minlength=nbins_used), out=bin_start[1:])
    e_slot = np.arange(len(eb)) - bin_start[eb]
    if len(e_slot) and e_slot.max() >= CAP:
        raise RuntimeError("edge overflow in a bin")

    rows_idx = eb * CAP + e_slot
    tidx = np.zeros(NBINS * CAP, np.int64)
    tidx[rows_idx] = ec
    t_all = support_bf[tidx]  # [NBINS*CAP, OUT_F] bf16 (pad rows killed by v=0)
    w_all = np.zeros(NBINS * CAP, np.float32)
    w_all[rows_idx] = ew
    v_all = np.zeros(NBINS * CAP, np.float32)
    v_all[rows_idx] = ev

    # swizzle: chunk rows -> partitions; stream = [w | v | T] per slab
    wswz = w_all.astype(BF16_NP).reshape(NSLABS, SLAB_CHUNKS, 128).transpose(0, 2, 1)
    vswz = v_all.astype(BF16_NP).reshape(NSLABS, SLAB_CHUNKS, 128).transpose(0, 2, 1)
    tswz = (
        t_all.reshape(NSLABS, SLAB_CHUNKS, 128, OUT_F)
        .transpose(0, 2, 1, 3)
        .reshape(NSLABS, 128, SLAB_CHUNKS * OUT_F)
    )
    stream = np.ascontiguousarray(np.concatenate([wswz, vswz, tswz], axis=2))
    return stream, destmap


def kernel(X_input, adj_row, adj_col, adj_val, W, bias):
    X_input = np.asarray(X_input, np.float32)
    adj_row = np.asarray(adj_row).astype(np.int64)
    adj_col = np.asarray(adj_col).astype(np.int64)
    adj_val = np.asarray(adj_val, np.float32)
    W = np.asarray(W, np.float32)
    bias = np.asarray(bias, np.float32)

    # ---- launch 1: support shards (bf16)
    nc1 = build_support_program()
    w_bf = np.ascontiguousarray(W.astype(BF16_NP).reshape(2, 128, OUT_F))
    in_maps1 = []
    for c in range(NCORES):
        sl = np.zeros((ROWS_PAD, IN_F), np.float32)
        lo = c * D_PER_CORE
        sl[:D_PER_CORE] = X_input[lo : lo + D_PER_CORE]
        xt = np.ascontiguousarray(
            sl.T.astype(BF16_NP).reshape(2, 128, ROWS_PAD)
        )
        in_maps1.append({"xt": xt, "w": w_bf})
    res1 = run_bass_kernel_spmd(nc1, in_maps1, list(range(NCORES)))
    kernel.last_res1 = res1
    shards = []
    for c in range(NCORES):
        s = res1.results[c]["sup"]  # [128, ROWS_PAD] bf16 = support^T
        shards.append(s.T[:D_PER_CORE])
    support_bf = np.ascontiguousarray(np.concatenate(shards, axis=0)).astype(BF16_NP)

    # ---- host packing
    iota_arr = np.ascontiguousarray(
        np.broadcast_to(np.arange(W_G, dtype=np.float32), (128, 1, W_G))
    ).astype(BF16_NP)
    core_of = adj_row // D_PER_CORE
    in_maps2 = []
    destmaps = []
    for c in range(NCORES):
        m = core_of == c
        stream, destmap = _pack_core(
            adj_row[m] - c * D_PER_CORE,
            adj_col[m],
            adj_val[m],
            support_bf,
        )
        destmaps.append(destmap)
        in_maps2.append({"stream": stream, "iota": iota_arr})

    # ---- launch 2
    nc2 = build_spmm_program()
    res2 = run_bass_kernel_spmd(nc2, in_maps2, list(range(NCORES)))
    kernel.last_res2 = res2
    out = np.empty((N_NODES, OUT_F), np.float32)
    for c in range(NCORES):
        o = res2.results[c]["out"]  # [NSLABS, W_G, BINS_PER_SLAB*OUT_F] bf16
        # slot (bin, w) -> o[s, w, bi*OUT_F : ...] where bin = s*BINS_PER_SLAB+bi
        o = (
            o.reshape(NSLABS, W_G, BINS_PER_SLAB, OUT_F)
            .transpose(0, 2, 1, 3)
            .reshape(NBINS * W_G, OUT_F)
        )
        dm = destmaps[c]
        valid = dm >= 0
        shard = np.zeros((D_PER_CORE, OUT_F), np.float32)
        np.add.at(shard, dm[valid], o[valid].astype(np.float32))
        shard += bias
        out[c * D_PER_CORE : (c + 1) * D_PER_CORE] = shard
    return out


# revision 41
# speedup vs baseline: 18.4737x; 1.0588x over previous
"""GCN layer (X @ W, then COO spmm scatter-add by dest, + bias) on 8 trn2 cores.

Strategy (dest-sharded, per sharding hint):
  Launch 1 (SPMD): core c computes support shard = X[c*12500:(c+1)*12500] @ W
    in bf16 (fp32 PSUM accumulate). Host pre-transposes X so the contraction
    dim lands on partitions.
  Host: assembles full support (bf16); packs each core's 12500 dest nodes
    into bins of <=32 dests and <=511 edges (next-fit-decreasing by degree).
    Per bin: a halo table T of the bin's unique source support rows (<=512
    rows, last row = bias) and a values matrix C [512, 32] with
    C[src_slot, dest_slot] = edge val (bias row = 1). out_bin = C^T @ T.
  Launch 2 (SPMD): pure sequential streaming -- no gathers. Per 64-chunk
    slab: stream T [128, 64*128] and C [128, 64*32] (host pre-swizzled so
    chunk rows land on partitions), one matmul per chunk accumulating each
    bin's 4 chunks in PSUM ([32 dests, 128 feats] per bin, 4 bins per PSUM
    bank), DVE-evacuate to bf16, DMA out per slab. Host unpermutes dest
    rows and casts to fp32.
"""

import numpy as np
import ml_dtypes

import concourse.bass as bass  # noqa: F401  (kept for parity with tile API)
import concourse.tile as tile
from concourse import bacc, mybir
from concourse.bass_utils import run_bass_kernel_spmd

BF16_NP = ml_dtypes.bfloat16

# ---------------- problem constants (hardcoded; kernel.py is self-contained)
N_NODES = 100000
N_EDGES = 1600000
IN_F = 256
OUT_F = 128
NCORES = 8

D_PER_CORE = N_NODES // NCORES  # 12500 dest nodes per core

# launch-1 (support matmul) geometry
ROWS_PAD = 12544  # 98 * 128
RTILES = ROWS_PAD // 128

# launch-2 (streamed halo spmm) geometry
W_G = 32  # dests per bin
CAP = 512  # table rows per bin (4 chunks of 128), one row per edge
EDGE_CAP = CAP  # <=512 edges per bin
CPB = CAP // 128  # chunks per bin = 4
SLAB_CHUNKS = 128  # chunks per slab (32 bins)
BINS_PER_SLAB = SLAB_CHUNKS // CPB  # 32
NSLABS = 13
NBINS = NSLABS * BINS_PER_SLAB  # 416
NCHUNKS = NBINS * CPB  # 1728

FP32 = mybir.dt.float32
BF16 = mybir.dt.bfloat16


def _new_nc():
    return bacc.Bacc("TRN2", target_bir_lowering=False, debug=False)


# ---------------- launch 1: support^T = (X_shard @ W)^T (bf16) ----------------
# Weights stationary (lhsT = W chunk), X columns stream (N=448 per matmul).
L1_N = 448
L1_TILES = ROWS_PAD // L1_N  # 28
L1_GRP = 4  # psum tiles in flight per k-sweep


def build_support_program():
    nc = _new_nc()
    xt = nc.declare_dram_parameter("xt", [2, 128, ROWS_PAD], BF16, isOutput=False)
    w = nc.declare_dram_parameter("w", [2, 128, OUT_F], BF16, isOutput=False)
    sup = nc.declare_dram_parameter("sup", [128, ROWS_PAD], BF16, isOutput=True)

    with tile.TileContext(nc) as tc:
        with (
            tc.tile_pool(name="xt_pool", bufs=1) as xt_pool,
            tc.tile_pool(name="w_pool", bufs=1) as w_pool,
            tc.tile_pool(name="out_pool", bufs=1) as out_pool,
            tc.tile_pool(name="ps_pool", bufs=2, space="PSUM") as ps_pool,
        ):
            w_t = w_pool.tile([128, 2, OUT_F], BF16)
            for k in range(2):
                nc.sync.dma_start(w_t[:, k, :], w[k])
            xt_t = xt_pool.tile([128, 2, ROWS_PAD], BF16)
            piece = L1_N * L1_GRP  # one group's worth of columns
            for h in range(ROWS_PAD // piece):
                for k in range(2):
                    nc.sync.dma_start(
                        xt_t[:, k, piece * h : piece * (h + 1)],
                        xt[k, :, piece * h : piece * (h + 1)],
                    )

            sup_buf = out_pool.tile([128, ROWS_PAD], BF16)
            ngrp = L1_TILES // L1_GRP
            gcols = L1_N * L1_GRP
            for g in range(ngrp):
                pss = [
                    ps_pool.tile([128, L1_N], FP32, space="PSUM", name=f"ps{t}")
                    for t in range(L1_GRP)
                ]
                for k in range(2):
                    for t in range(L1_GRP):
                        i = g * L1_GRP + t
                        nc.tensor.matmul(
                            out=pss[t][:],
                            lhsT=w_t[:, k, :],
                            rhs=xt_t[:, k, L1_N * i : L1_N * (i + 1)],
                            start=(k == 0),
                            stop=(k == 1),
                        )
                for t in range(L1_GRP):
                    i = g * L1_GRP + t
                    nc.vector.tensor_copy(
                        sup_buf[:, L1_N * i : L1_N * (i + 1)], pss[t][:]
                    )
                nc.scalar.dma_start(
                    sup[:, gcols * g : gcols * (g + 1)],
                    sup_buf[:, gcols * g : gcols * (g + 1)],
                )
    nc.compile()
    return nc


# ---------------- launch 2: streamed halo spmm ----------------
def build_spmm_program(real_chunks=NCHUNKS):
    nc = _new_nc()
    # per slab: [w: SLAB_CHUNKS cols][v: SLAB_CHUNKS cols][T: SLAB_CHUNKS*OUT_F]
    VOFF = SLAB_CHUNKS
    TOFF = 2 * SLAB_CHUNKS
    SCOLS = SLAB_CHUNKS * (OUT_F + 2)
    stream = nc.declare_dram_parameter(
        "stream", [NSLABS, 128, SCOLS], BF16, isOutput=False
    )
    iota = nc.declare_dram_parameter("iota", [128, 1, W_G], BF16, isOutput=False)
    out = nc.declare_dram_parameter(
        "out", [NSLABS, W_G, BINS_PER_SLAB * OUT_F], BF16, isOutput=True
    )

    groups_per_slab = BINS_PER_SLAB // 4  # 4 bins per PSUM bank
    NQ = 4  # T quarter-loads per slab
    QC = SLAB_CHUNKS // NQ  # chunks per quarter

    with tile.TileContext(nc) as tc:
        with (
            tc.tile_pool(name="const_pool", bufs=1) as const_pool,
            tc.tile_pool(name="s_pool", bufs=3) as s_pool,
            tc.tile_pool(name="c_pool", bufs=2) as c_pool,
            tc.tile_pool(name="o_pool", bufs=2) as o_pool,
            tc.tile_pool(name="ps_pool", bufs=4, space="PSUM") as ps_pool,
        ):
            iota_t = const_pool.tile([128, 1, W_G], BF16)
            nc.sync.dma_start(iota_t[:], iota[:])

            for s in range(NSLABS):
                # chunks of this slab that hold real bins (rest skipped)
                live = min(max(real_chunks - s * SLAB_CHUNKS, 0), SLAB_CHUNKS)
                if live == 0:
                    break
                st = s_pool.tile([128, SCOLS], BF16)
                nc.sync.dma_start(st[:, :TOFF], stream[s, :, :TOFF])  # w|v first
                cb = c_pool.tile([128, SLAB_CHUNKS, W_G], BF16)
                ceq = c_pool.tile([128, SLAB_CHUNKS, W_G], BF16)
                for qi in range(NQ):
                    lo, hi = QC * qi, QC * (qi + 1)
                    if lo * OUT_F >= live * OUT_F:
                        break
                    tl = min(hi, live)
                    nc.sync.dma_start(
                        st[:, TOFF + OUT_F * lo : TOFF + OUT_F * tl],
                        stream[s, :, TOFF + OUT_F * lo : TOFF + OUT_F * tl],
                    )
                    # build C for this quarter: cb = (w == iota) * v
                    nc.vector.tensor_tensor(
                        out=ceq[:, lo:hi, :],
                        in0=st[:, lo:hi].to_broadcast([128, QC, W_G]),
                        in1=iota_t[:].to_broadcast([128, QC, W_G]),
                        op=mybir.AluOpType.is_equal,
                    )
                    nc.vector.tensor_tensor(
                        out=cb[:, lo:hi, :],
                        in0=ceq[:, lo:hi, :],
                        in1=st[:, VOFF + lo : VOFF + hi].to_broadcast([128, QC, W_G]),
                        op=mybir.AluOpType.mult,
                    )

                o_t = o_pool.tile([W_G, BINS_PER_SLAB * OUT_F], BF16)
                live_groups = (live + 4 * CPB - 1) // (4 * CPB)
                for g in range(live_groups):
                    ps = ps_pool.tile([W_G, 4 * OUT_F], FP32, space="PSUM")
                    for j in range(4):  # bin within group
                        b = g * 4 + j
                        if b * CPB >= live:
                            break
                        for k in range(CPB):
                            c = b * CPB + k
                            nc.tensor.matmul(
                                out=ps[:, OUT_F * j : OUT_F * (j + 1)],
                                lhsT=cb[:, c, :],
                                rhs=st[:, TOFF + OUT_F * c : TOFF + OUT_F * (c + 1)],
                                start=(k == 0),
                                stop=(k == CPB - 1),
                            )
                    nc.scalar.copy(
                        out=o_t[:, 4 * OUT_F * g : 4 * OUT_F * (g + 1)],
                        in_=ps[:],
                    )
                nc.scalar.dma_start(
                    out[s, :, : 4 * OUT_F * live_groups],
                    o_t[:, : 4 * OUT_F * live_groups],
                )
    nc.compile()
    return nc


# ---------------- host-side packing ----------------
def _pack_core(rows_c, cols_c, vals_c, support_bf):
    """Pack one core's edges into (tswz, cswz, destmap).

    rows_c: local dest ids [0, 12500); cols_c: global src ids; vals_c: f32.
    Returns stream [NSLABS,128,SLAB_CHUNKS*(OUT_F+2)] bf16 ([w|v|T] per
    slab) and destmap [NBINS*W_G] int64 (-1 for unused slots, multiple
    slots may map to one dest -- host sums, then de-dupes bias).
    """
    deg = np.bincount(rows_c, minlength=D_PER_CORE)

    # balanced two-pointer binning with dest splitting: <=32 slots and
    # <=EDGE_CAP edges per bin. Take from the high-degree end when the
    # remaining capacity-per-slot exceeds the average degree, else from
    # the low end; a dest whose edges overflow the bin is split across
    # bins (host sums the partial outputs; bias counted once).
    order = np.argsort(-deg, kind="stable")
    degs = deg[order].astype(np.int64)
    n = len(order)
    avg = degs.sum() / D_PER_CORE
    piece_dest, piece_bin, piece_w, piece_take, piece_first = [], [], [], [], []
    i, j = 0, n - 1
    rem_front = int(degs[0])
    front_first = True
    b = 0

    def place(d, w, take, first):
        piece_dest.append(d)
        piece_bin.append(b)
        piece_w.append(w)
        piece_take.append(take)
        piece_first.append(first)

    while i <= j:
        slots, fill = 0, 0
        while slots < W_G and i <= j:
            cap = EDGE_CAP - fill
            if i == j:
                take = min(rem_front, cap)
                if take == 0 and rem_front > 0:
                    break
                place(int(order[i]), slots, take, front_first)
                front_first = False
                slots += 1
                fill += take
                rem_front -= take
                if rem_front == 0:
                    i += 1
                continue
            if (cap / (W_G - slots)) >= avg:
                take = min(rem_front, cap)
                if take < rem_front and take == 0:
                    break
                place(int(order[i]), slots, take, front_first)
                front_first = False
                slots += 1
                fill += take
                rem_front -= take
                if rem_front == 0:
                    i += 1
                    rem_front = int(degs[i]) if i < n else 0
                    front_first = True
            else:
                db = int(degs[j])
                if db <= cap:
                    place(int(order[j]), slots, db, True)
                    slots += 1
                    fill += db
                    j -= 1
                else:
                    if cap == 0:
                        break
                    take = min(rem_front, cap)
                    place(int(order[i]), slots, take, front_first)
                    front_first = False
                    slots += 1
                    fill += take
                    rem_front -= take
                    if rem_front == 0:
                        i += 1
                        rem_front = int(degs[i]) if i < n else 0
                        front_first = True
        b += 1
    nbins_used = b
    if nbins_used > NBINS:
        raise RuntimeError(f"bin overflow: {nbins_used} > {NBINS}")
    piece_dest = np.array(piece_dest, np.int64)
    piece_bin = np.array(piece_bin, np.int64)
    piece_w = np.array(piece_w, np.int64)
    piece_take = np.array(piece_take, np.int64)
    piece_first = np.array(piece_first, bool)

    destmap = np.full(NBINS * W_G, -1, np.int64)
    destmap[piece_bin * W_G + piece_w] = piece_dest

    # per-edge piece: edges sorted by dest; rank within dest selects piece
    order_d = np.argsort(rows_c, kind="stable")
    dstart = np.zeros(D_PER_CORE + 1, np.int64)
    np.cumsum(deg, out=dstart[1:])
    rank = np.arange(len(rows_c)) - dstart[rows_c[order_d]]
    # piece boundaries per dest: order pieces by (dest, first-come)
    po = np.lexsort((np.arange(len(piece_dest)), piece_dest))
    p_d = piece_dest[po]
    p_take = piece_take[po]
    p_off = np.zeros(len(po), np.int64)
    newd = np.empty(len(po), bool)
    newd[0] = True
    np.not_equal(p_d[1:], p_d[:-1], out=newd[1:])
    csum = np.cumsum(p_take) - p_take
    base = np.where(newd, csum, 0)
    np.maximum.accumulate(base, out=base)
    p_off = csum - base  # start rank of each piece within its dest
    # map each edge (dest, rank) -> piece index via searchsorted per dest
    pstart_of_dest = np.zeros(D_PER_CORE + 1, np.int64)
    np.cumsum(np.bincount(p_d, minlength=D_PER_CORE), out=pstart_of_dest[1:])
    ed = rows_c[order_d]
    lo = pstart_of_dest[ed]
    hi = pstart_of_dest[ed + 1]
    # pieces per dest are tiny (1-2); resolve by comparing rank to offsets
    pidx = lo.copy()
    multi = hi - lo > 1
    if multi.any():
        # iterate piece levels (max pieces per dest is small)
        maxp = int((hi - lo).max())
        for lvl in range(1, maxp):
            cand = lo + lvl
            ok = (cand < hi) & (rank >= p_off[np.minimum(cand, len(p_off) - 1)])
            pidx = np.where(ok, cand, pidx)
    e_bin = np.empty(len(rows_c), np.int64)
    e_w = np.empty(len(rows_c), np.int64)
    e_bin[order_d] = piece_bin[po][pidx]
    e_w[order_d] = piece_w[po][pidx]

    # one table slot per edge: sort edges by bin, slot = rank within bin
    order_e = np.argsort(e_bin, kind="stable")
    eb = e_bin[order_e]
    ec = cols_c[order_e]
    ew = e_w[order_e]
    ev = vals_c[order_e]
    bin_start = np.zeros(nbins_used + 1, np.int64)
    np.cumsum(np.bincount(eb, minlength=nbins_used), out=bin_start[1:])
    e_slot = np.arange(len(eb)) - bin_start[eb]
    if len(e_slot) and e_slot.max() >= CAP:
        raise RuntimeError("edge overflow in a bin")

    rows_idx = eb * CAP + e_slot
    tidx = np.zeros(NBINS * CAP, np.int64)
    tidx[rows_idx] = ec
    t_all = support_bf[tidx]  # [NBINS*CAP, OUT_F] bf16 (pad rows killed by v=0)
    w_all = np.zeros(NBINS * CAP, np.float32)
    w_all[rows_idx] = ew
    v_all = np.zeros(NBINS * CAP, np.float32)
    v_all[rows_idx] = ev

    # swizzle: chunk rows -> partitions; stream = [w | v | T] per slab
    wswz = w_all.astype(BF16_NP).reshape(NSLABS, SLAB_CHUNKS, 128).transpose(0, 2, 1)
    vswz = v_all.astype(BF16_NP).reshape(NSLABS, SLAB_CHUNKS, 128).transpose(0, 2, 1)
    tswz = (
        t_all.reshape(NSLABS, SLAB_CHUNKS, 128, OUT_F)
        .transpose(0, 2, 1, 3)
        .reshape(NSLABS, 128, SLAB_CHUNKS * OUT_F)
    )
    stream = np.ascontiguousarray(np.concatenate([wswz, vswz, tswz], axis=2))
    return stream, destmap


def kernel(X_input, adj_row, adj_col, adj_val, W, bias):
    X_input = np.asarray(X_input, np.float32)
    adj_row = np.asarray(adj_row).astype(np.int64)
    adj_col = np.asarray(adj_col).astype(np.int64)
    adj_val = np.asarray(adj_val, np.float32)
    W = np.asarray(W, np.float32)
    bias = np.asarray(bias, np.float32)

    # ---- launch 1: support shards (bf16)
    nc1 = build_support_program()
    w_bf = np.ascontiguousarray(W.astype(BF16_NP).reshape(2, 128, OUT_F))
    in_maps1 = []
    for c in range(NCORES):
        sl = np.zeros((ROWS_PAD, IN_F), np.float32)
        lo = c * D_PER_CORE
        sl[:D_PER_CORE] = X_input[lo : lo + D_PER_CORE]
        xt = np.ascontiguousarray(
            sl.T.astype(BF16_NP).reshape(2, 128, ROWS_PAD)
        )
        in_maps1.append({"xt": xt, "w": w_bf})
    res1 = run_bass_kernel_spmd(nc1, in_maps1, list(range(NCORES)))
    kernel.last_res1 = res1
    shards = []
    for c in range(NCORES):
        s = res1.results[c]["sup"]  # [128, ROWS_PAD] bf16 = support^T
        shards.append(s.T[:D_PER_CORE])
    support_bf = np.ascontiguousarray(np.concatenate(shards, axis=0)).astype(BF16_NP)

    # ---- host packing
    iota_arr = np.ascontiguousarray(
        np.broadcast_to(np.arange(W_G, dtype=np.float32), (128, 1, W_G))
    ).astype(BF16_NP)
    core_of = adj_row // D_PER_CORE
    in_maps2 = []
    destmaps = []
    for c in range(NCORES):
        m = core_of == c
        stream, destmap = _pack_core(
            adj_row[m] - c * D_PER_CORE,
            adj_col[m],
            adj_val[m],
            support_bf,
        )
        destmaps.append(destmap)
        in_maps2.append({"stream": stream, "iota": iota_arr})

    # ---- launch 2 (compiled for the worst-case real bin count)
    maxbins = max(
        int((dm.reshape(NBINS, W_G) >= 0).any(axis=1).sum()) for dm in destmaps
    )
    nc2 = build_spmm_program(real_chunks=maxbins * CPB)
    res2 = run_bass_kernel_spmd(nc2, in_maps2, list(range(NCORES)))
    kernel.last_res2 = res2
    out = np.empty((N_NODES, OUT_F), np.float32)
    for c in range(NCORES):
        o = res2.results[c]["out"]  # [NSLABS, W_G, BINS_PER_SLAB*OUT_F] bf16
        # slot (bin, w) -> o[s, w, bi*OUT_F : ...] where bin = s*BINS_PER_SLAB+bi
        o = (
            o.reshape(NSLABS, W_G, BINS_PER_SLAB, OUT_F)
            .transpose(0, 2, 1, 3)
            .reshape(NBINS * W_G, OUT_F)
        )
        dm = destmaps[c]
        valid = dm >= 0
        shard = np.zeros((D_PER_CORE, OUT_F), np.float32)
        np.add.at(shard, dm[valid], o[valid].astype(np.float32))
        shard += bias
        out[c * D_PER_CORE : (c + 1) * D_PER_CORE] = shard
    return out


# revision 42
# speedup vs baseline: 19.0409x; 1.0307x over previous
"""GCN layer (X @ W, then COO spmm scatter-add by dest, + bias) on 8 trn2 cores.

Strategy (dest-sharded, per sharding hint):
  Launch 1 (SPMD): core c computes support shard = X[c*12500:(c+1)*12500] @ W
    in bf16 (fp32 PSUM accumulate). Host pre-transposes X so the contraction
    dim lands on partitions.
  Host: assembles full support (bf16); packs each core's 12500 dest nodes
    into bins of <=32 dests and <=511 edges (next-fit-decreasing by degree).
    Per bin: a halo table T of the bin's unique source support rows (<=512
    rows, last row = bias) and a values matrix C [512, 32] with
    C[src_slot, dest_slot] = edge val (bias row = 1). out_bin = C^T @ T.
  Launch 2 (SPMD): pure sequential streaming -- no gathers. Per 64-chunk
    slab: stream T [128, 64*128] and C [128, 64*32] (host pre-swizzled so
    chunk rows land on partitions), one matmul per chunk accumulating each
    bin's 4 chunks in PSUM ([32 dests, 128 feats] per bin, 4 bins per PSUM
    bank), DVE-evacuate to bf16, DMA out per slab. Host unpermutes dest
    rows and casts to fp32.
"""

import numpy as np
import ml_dtypes

import concourse.bass as bass  # noqa: F401  (kept for parity with tile API)
import concourse.tile as tile
from concourse import bacc, mybir
from concourse.bass_utils import run_bass_kernel_spmd

BF16_NP = ml_dtypes.bfloat16

# ---------------- problem constants (hardcoded; kernel.py is self-contained)
N_NODES = 100000
N_EDGES = 1600000
IN_F = 256
OUT_F = 128
NCORES = 8

D_PER_CORE = N_NODES // NCORES  # 12500 dest nodes per core

# launch-1 (support matmul) geometry
ROWS_PAD = 12544  # 98 * 128
RTILES = ROWS_PAD // 128

# launch-2 (streamed halo spmm) geometry
W_G = 32  # dests per bin
CAP = 512  # table rows per bin (4 chunks of 128), one row per edge
EDGE_CAP = CAP  # <=512 edges per bin
CPB = CAP // 128  # chunks per bin = 4
SLAB_CHUNKS = 128  # chunks per slab (32 bins)
BINS_PER_SLAB = SLAB_CHUNKS // CPB  # 32
NSLABS = 13
NBINS = NSLABS * BINS_PER_SLAB  # 416
NCHUNKS = NBINS * CPB  # 1728

FP32 = mybir.dt.float32
BF16 = mybir.dt.bfloat16


def _new_nc():
    return bacc.Bacc("TRN2", target_bir_lowering=False, debug=False)


# ---------------- launch 1: support^T = (X_shard @ W)^T (bf16) ----------------
# Weights stationary (lhsT = W chunk), X columns stream (N=448 per matmul).
L1_N = 448
L1_TILES = ROWS_PAD // L1_N  # 28
L1_GRP = 4  # psum tiles in flight per k-sweep


def build_support_program():
    nc = _new_nc()
    xt = nc.declare_dram_parameter("xt", [2, 128, ROWS_PAD], BF16, isOutput=False)
    w = nc.declare_dram_parameter("w", [2, 128, OUT_F], BF16, isOutput=False)
    sup = nc.declare_dram_parameter("sup", [128, ROWS_PAD], BF16, isOutput=True)

    with tile.TileContext(nc) as tc:
        with (
            tc.tile_pool(name="xt_pool", bufs=1) as xt_pool,
            tc.tile_pool(name="w_pool", bufs=1) as w_pool,
            tc.tile_pool(name="out_pool", bufs=1) as out_pool,
            tc.tile_pool(name="ps_pool", bufs=2, space="PSUM") as ps_pool,
        ):
            w_t = w_pool.tile([128, 2, OUT_F], BF16)
            for k in range(2):
                nc.sync.dma_start(w_t[:, k, :], w[k])
            xt_t = xt_pool.tile([128, 2, ROWS_PAD], BF16)
            piece = L1_N * L1_GRP  # one group's worth of columns
            for h in range(ROWS_PAD // piece):
                for k in range(2):
                    eng = nc.sync if k == 0 else nc.scalar
                    eng.dma_start(
                        xt_t[:, k, piece * h : piece * (h + 1)],
                        xt[k, :, piece * h : piece * (h + 1)],
                    )

            sup_buf = out_pool.tile([128, ROWS_PAD], BF16)
            ngrp = L1_TILES // L1_GRP
            gcols = L1_N * L1_GRP
            for g in range(ngrp):
                pss = [
                    ps_pool.tile([128, L1_N], FP32, space="PSUM", name=f"ps{t}")
                    for t in range(L1_GRP)
                ]
                for k in range(2):
                    for t in range(L1_GRP):
                        i = g * L1_GRP + t
                        nc.tensor.matmul(
                            out=pss[t][:],
                            lhsT=w_t[:, k, :],
                            rhs=xt_t[:, k, L1_N * i : L1_N * (i + 1)],
                            start=(k == 0),
                            stop=(k == 1),
                        )
                for t in range(L1_GRP):
                    i = g * L1_GRP + t
                    nc.vector.tensor_copy(
                        sup_buf[:, L1_N * i : L1_N * (i + 1)], pss[t][:]
                    )
                nc.scalar.dma_start(
                    sup[:, gcols * g : gcols * (g + 1)],
                    sup_buf[:, gcols * g : gcols * (g + 1)],
                )
    nc.compile()
    return nc


# ---------------- launch 2: streamed halo spmm ----------------
def build_spmm_program(real_chunks=NCHUNKS):
    nc = _new_nc()
    # per slab: [w: SLAB_CHUNKS cols][v: SLAB_CHUNKS cols][T: SLAB_CHUNKS*OUT_F]
    VOFF = SLAB_CHUNKS
    TOFF = 2 * SLAB_CHUNKS
    SCOLS = SLAB_CHUNKS * (OUT_F + 2)
    stream = nc.declare_dram_parameter(
        "stream", [NSLABS, 128, SCOLS], BF16, isOutput=False
    )
    iota = nc.declare_dram_parameter("iota", [128, 1, W_G], BF16, isOutput=False)
    out = nc.declare_dram_parameter(
        "out", [NSLABS, W_G, BINS_PER_SLAB * OUT_F], BF16, isOutput=True
    )

    groups_per_slab = BINS_PER_SLAB // 4  # 4 bins per PSUM bank
    NQ = 4  # T quarter-loads per slab
    QC = SLAB_CHUNKS // NQ  # chunks per quarter

    with tile.TileContext(nc) as tc:
        with (
            tc.tile_pool(name="const_pool", bufs=1) as const_pool,
            tc.tile_pool(name="s_pool", bufs=3) as s_pool,
            tc.tile_pool(name="c_pool", bufs=2) as c_pool,
            tc.tile_pool(name="o_pool", bufs=2) as o_pool,
            tc.tile_pool(name="ps_pool", bufs=4, space="PSUM") as ps_pool,
        ):
            iota_t = const_pool.tile([128, 1, W_G], BF16)
            nc.sync.dma_start(iota_t[:], iota[:])

            for s in range(NSLABS):
                # chunks of this slab that hold real bins (rest skipped)
                live = min(max(real_chunks - s * SLAB_CHUNKS, 0), SLAB_CHUNKS)
                if live == 0:
                    break
                st = s_pool.tile([128, SCOLS], BF16)
                nc.sync.dma_start(st[:, :TOFF], stream[s, :, :TOFF])  # w|v first
                cb = c_pool.tile([128, SLAB_CHUNKS, W_G], BF16)
                ceq = c_pool.tile([128, SLAB_CHUNKS, W_G], BF16)
                for qi in range(NQ):
                    lo, hi = QC * qi, QC * (qi + 1)
                    if lo * OUT_F >= live * OUT_F:
                        break
                    tl = min(hi, live)
                    nc.sync.dma_start(
                        st[:, TOFF + OUT_F * lo : TOFF + OUT_F * tl],
                        stream[s, :, TOFF + OUT_F * lo : TOFF + OUT_F * tl],
                    )
                    # build C for this quarter: cb = (w == iota) * v
                    nc.vector.tensor_tensor(
                        out=ceq[:, lo:hi, :],
                        in0=st[:, lo:hi].to_broadcast([128, QC, W_G]),
                        in1=iota_t[:].to_broadcast([128, QC, W_G]),
                        op=mybir.AluOpType.is_equal,
                    )
                    nc.vector.tensor_tensor(
                        out=cb[:, lo:hi, :],
                        in0=ceq[:, lo:hi, :],
                        in1=st[:, VOFF + lo : VOFF + hi].to_broadcast([128, QC, W_G]),
                        op=mybir.AluOpType.mult,
                    )

                o_t = o_pool.tile([W_G, BINS_PER_SLAB * OUT_F], BF16)
                live_groups = (live + 4 * CPB - 1) // (4 * CPB)
                for g in range(live_groups):
                    ps = ps_pool.tile([W_G, 4 * OUT_F], FP32, space="PSUM")
                    for j in range(4):  # bin within group
                        b = g * 4 + j
                        if b * CPB >= live:
                            break
                        for k in range(CPB):
                            c = b * CPB + k
                            nc.tensor.matmul(
                                out=ps[:, OUT_F * j : OUT_F * (j + 1)],
                                lhsT=cb[:, c, :],
                                rhs=st[:, TOFF + OUT_F * c : TOFF + OUT_F * (c + 1)],
                                start=(k == 0),
                                stop=(k == CPB - 1),
                            )
                    nc.scalar.copy(
                        out=o_t[:, 4 * OUT_F * g : 4 * OUT_F * (g + 1)],
                        in_=ps[:],
                    )
                nc.scalar.dma_start(
                    out[s, :, : 4 * OUT_F * live_groups],
                    o_t[:, : 4 * OUT_F * live_groups],
                )
    nc.compile()
    return nc


# ---------------- host-side packing ----------------
def _pack_core(rows_c, cols_c, vals_c, support_bf):
    """Pack one core's edges into (tswz, cswz, destmap).

    rows_c: local dest ids [0, 12500); cols_c: global src ids; vals_c: f32.
    Returns stream [NSLABS,128,SLAB_CHUNKS*(OUT_F+2)] bf16 ([w|v|T] per
    slab) and destmap [NBINS*W_G] int64 (-1 for unused slots, multiple
    slots may map to one dest -- host sums, then de-dupes bias).
    """
    deg = np.bincount(rows_c, minlength=D_PER_CORE)

    # balanced two-pointer binning with dest splitting: <=32 slots and
    # <=EDGE_CAP edges per bin. Take from the high-degree end when the
    # remaining capacity-per-slot exceeds the average degree, else from
    # the low end; a dest whose edges overflow the bin is split across
    # bins (host sums the partial outputs; bias counted once).
    order = np.argsort(-deg, kind="stable")
    degs = deg[order].astype(np.int64)
    n = len(order)
    avg = degs.sum() / D_PER_CORE
    piece_dest, piece_bin, piece_w, piece_take, piece_first = [], [], [], [], []
    i, j = 0, n - 1
    rem_front = int(degs[0])
    front_first = True
    b = 0

    def place(d, w, take, first):
        piece_dest.append(d)
        piece_bin.append(b)
        piece_w.append(w)
        piece_take.append(take)
        piece_first.append(first)

    while i <= j:
        slots, fill = 0, 0
        while slots < W_G and i <= j:
            cap = EDGE_CAP - fill
            if i == j:
                take = min(rem_front, cap)
                if take == 0 and rem_front > 0:
                    break
                place(int(order[i]), slots, take, front_first)
                front_first = False
                slots += 1
                fill += take
                rem_front -= take
                if rem_front == 0:
                    i += 1
                continue
            if (cap / (W_G - slots)) >= avg:
                take = min(rem_front, cap)
                if take < rem_front and take == 0:
                    break
                place(int(order[i]), slots, take, front_first)
                front_first = False
                slots += 1
                fill += take
                rem_front -= take
                if rem_front == 0:
                    i += 1
                    rem_front = int(degs[i]) if i < n else 0
                    front_first = True
            else:
                db = int(degs[j])
                if db <= cap:
                    place(int(order[j]), slots, db, True)
                    slots += 1
                    fill += db
                    j -= 1
                else:
                    if cap == 0:
                        break
                    take = min(rem_front, cap)
                    place(int(order[i]), slots, take, front_first)
                    front_first = False
                    slots += 1
                    fill += take
                    rem_front -= take
                    if rem_front == 0:
                        i += 1
                        rem_front = int(degs[i]) if i < n else 0
                        front_first = True
        b += 1
    nbins_used = b
    if nbins_used > NBINS:
        raise RuntimeError(f"bin overflow: {nbins_used} > {NBINS}")
    piece_dest = np.array(piece_dest, np.int64)
    piece_bin = np.array(piece_bin, np.int64)
    piece_w = np.array(piece_w, np.int64)
    piece_take = np.array(piece_take, np.int64)
    piece_first = np.array(piece_first, bool)

    destmap = np.full(NBINS * W_G, -1, np.int64)
    destmap[piece_bin * W_G + piece_w] = piece_dest

    # per-edge piece: edges sorted by dest; rank within dest selects piece
    order_d = np.argsort(rows_c, kind="stable")
    dstart = np.zeros(D_PER_CORE + 1, np.int64)
    np.cumsum(deg, out=dstart[1:])
    rank = np.arange(len(rows_c)) - dstart[rows_c[order_d]]
    # piece boundaries per dest: order pieces by (dest, first-come)
    po = np.lexsort((np.arange(len(piece_dest)), piece_dest))
    p_d = piece_dest[po]
    p_take = piece_take[po]
    p_off = np.zeros(len(po), np.int64)
    newd = np.empty(len(po), bool)
    newd[0] = True
    np.not_equal(p_d[1:], p_d[:-1], out=newd[1:])
    csum = np.cumsum(p_take) - p_take
    base = np.where(newd, csum, 0)
    np.maximum.accumulate(base, out=base)
    p_off = csum - base  # start rank of each piece within its dest
    # map each edge (dest, rank) -> piece index via searchsorted per dest
    pstart_of_dest = np.zeros(D_PER_CORE + 1, np.int64)
    np.cumsum(np.bincount(p_d, minlength=D_PER_CORE), out=pstart_of_dest[1:])
    ed = rows_c[order_d]
    lo = pstart_of_dest[ed]
    hi = pstart_of_dest[ed + 1]
    # pieces per dest are tiny (1-2); resolve by comparing rank to offsets
    pidx = lo.copy()
    multi = hi - lo > 1
    if multi.any():
        # iterate piece levels (max pieces per dest is small)
        maxp = int((hi - lo).max())
        for lvl in range(1, maxp):
            cand = lo + lvl
            ok = (cand < hi) & (rank >= p_off[np.minimum(cand, len(p_off) - 1)])
            pidx = np.where(ok, cand, pidx)
    e_bin = np.empty(len(rows_c), np.int64)
    e_w = np.empty(len(rows_c), np.int64)
    e_bin[order_d] = piece_bin[po][pidx]
    e_w[order_d] = piece_w[po][pidx]

    # one table slot per edge: sort edges by bin, slot = rank within bin
    order_e = np.argsort(e_bin, kind="stable")
    eb = e_bin[order_e]
    ec = cols_c[order_e]
    ew = e_w[order_e]
    ev = vals_c[order_e]
    bin_start = np.zeros(nbins_used + 1, np.int64)
    np.cumsum(np.bincount(eb, minlength=nbins_used), out=bin_start[1:])
    e_slot = np.arange(len(eb)) - bin_start[eb]
    if len(e_slot) and e_slot.max() >= CAP:
        raise RuntimeError("edge overflow in a bin")

    rows_idx = eb * CAP + e_slot
    tidx = np.zeros(NBINS * CAP, np.int64)
    tidx[rows_idx] = ec
    t_all = support_bf[tidx]  # [NBINS*CAP, OUT_F] bf16 (pad rows killed by v=0)
    w_all = np.zeros(NBINS * CAP, np.float32)
    w_all[rows_idx] = ew
    v_all = np.zeros(NBINS * CAP, np.float32)
    v_all[rows_idx] = ev

    # swizzle: chunk rows -> partitions; stream = [w | v | T] per slab
    wswz = w_all.astype(BF16_NP).reshape(NSLABS, SLAB_CHUNKS, 128).transpose(0, 2, 1)
    vswz = v_all.astype(BF16_NP).reshape(NSLABS, SLAB_CHUNKS, 128).transpose(0, 2, 1)
    tswz = (
        t_all.reshape(NSLABS, SLAB_CHUNKS, 128, OUT_F)
        .transpose(0, 2, 1, 3)
        .reshape(NSLABS, 128, SLAB_CHUNKS * OUT_F)
    )
    stream = np.ascontiguousarray(np.concatenate([wswz, vswz, tswz], axis=2))
    return stream, destmap


def kernel(X_input, adj_row, adj_col, adj_val, W, bias):
    X_input = np.asarray(X_input, np.float32)
    adj_row = np.asarray(adj_row).astype(np.int64)
    adj_col = np.asarray(adj_col).astype(np.int64)
    adj_val = np.asarray(adj_val, np.float32)
    W = np.asarray(W, np.float32)
    bias = np.asarray(bias, np.float32)

    # ---- launch 1: support shards (bf16)
    nc1 = build_support_program()
    w_bf = np.ascontiguousarray(W.astype(BF16_NP).reshape(2, 128, OUT_F))
    in_maps1 = []
    for c in range(NCORES):
        sl = np.zeros((ROWS_PAD, IN_F), np.float32)
        lo = c * D_PER_CORE
        sl[:D_PER_CORE] = X_input[lo : lo + D_PER_CORE]
        xt = np.ascontiguousarray(
            sl.T.astype(BF16_NP).reshape(2, 128, ROWS_PAD)
        )
        in_maps1.append({"xt": xt, "w": w_bf})
    res1 = run_bass_kernel_spmd(nc1, in_maps1, list(range(NCORES)))
    kernel.last_res1 = res1
    shards = []
    for c in range(NCORES):
        s = res1.results[c]["sup"]  # [128, ROWS_PAD] bf16 = support^T
        shards.append(s.T[:D_PER_CORE])
    support_bf = np.ascontiguousarray(np.concatenate(shards, axis=0)).astype(BF16_NP)

    # ---- host packing
    iota_arr = np.ascontiguousarray(
        np.broadcast_to(np.arange(W_G, dtype=np.float32), (128, 1, W_G))
    ).astype(BF16_NP)
    core_of = adj_row // D_PER_CORE
    in_maps2 = []
    destmaps = []
    for c in range(NCORES):
        m = core_of == c
        stream, destmap = _pack_core(
            adj_row[m] - c * D_PER_CORE,
            adj_col[m],
            adj_val[m],
            support_bf,
        )
        destmaps.append(destmap)
        in_maps2.append({"stream": stream, "iota": iota_arr})

    # ---- launch 2 (compiled for the worst-case real bin count)
    maxbins = max(
        int((dm.reshape(NBINS, W_G) >= 0).any(axis=1).sum()) for dm in destmaps
    )
    nc2 = build_spmm_program(real_chunks=maxbins * CPB)
    res2 = run_bass_kernel_spmd(nc2, in_maps2, list(range(NCORES)))
    kernel.last_res2 = res2
    out = np.empty((N_NODES, OUT_F), np.float32)
    for c in range(NCORES):
        o = res2.results[c]["out"]  # [NSLABS, W_G, BINS_PER_SLAB*OUT_F] bf16
        # slot (bin, w) -> o[s, w, bi*OUT_F : ...] where bin = s*BINS_PER_SLAB+bi
        o = (
            o.reshape(NSLABS, W_G, BINS_PER_SLAB, OUT_F)
            .transpose(0, 2, 1, 3)
            .reshape(NBINS * W_G, OUT_F)
        )
        dm = destmaps[c]
        valid = dm >= 0
        shard = np.zeros((D_PER_CORE, OUT_F), np.float32)
        np.add.at(shard, dm[valid], o[valid].astype(np.float32))
        shard += bias
        out[c * D_PER_CORE : (c + 1) * D_PER_CORE] = shard
    return out


# revision 44
# speedup vs baseline: 19.1700x; 1.0068x over previous
"""GCN layer (X @ W, then COO spmm scatter-add by dest, + bias) on 8 trn2 cores.

Strategy (dest-sharded, per sharding hint; avoids per-edge DMA descriptors
entirely -- SWDGE dma_gather costs ~8ns/row of serialized GpSimd time, which
made a gather-based version Q7-bound at ~3.9ms):
  Launch 1 (SPMD): core c computes support^T shard = (X[c*12500:...] @ W)^T
    in bf16 (fp32 PSUM accumulate), W stationary in the PE array, X columns
    streaming 448 wide. Host pre-transposes X so the contraction dim lands
    on partitions.
  Host: assembles full support (bf16); packs each core's 12500 dest nodes
    into bins of <=32 dest slots and <=512 edges (balanced two-pointer over
    the degree-sorted dests, splitting a dest across bins when it overflows;
    host later sums split partial outputs). Each edge gets one table slot:
    T row = support[src] (the halo exchange of the sharding hint), plus
    (w, v) = (dest slot, edge val) metadata.
  Launch 2 (SPMD): pure sequential streaming -- no gathers. Per 128-chunk
    slab: one [w|v] load then 4 quarter T loads (host pre-swizzled so chunk
    rows land on partitions); DVE builds the scatter matrix on-chip,
    C = (w == iota) * v [128, chunk, 32]; one matmul per chunk
    (lhsT=C chunk, rhs=T chunk) accumulating each bin's 4 chunks in PSUM
    ([32 dests, 128 feats] per bin, 4 bins per PSUM bank); scalar-engine
    (ACT) evacuates PSUM to bf16; out store per slab on the scalar HWDGE
    ring so the sync ring stays a pure load stream. Host sums dest slots,
    adds bias once, casts to fp32.
"""

import numpy as np
import ml_dtypes

import concourse.tile as tile
from concourse import bacc, mybir
from concourse.bass_utils import run_bass_kernel_spmd

BF16_NP = ml_dtypes.bfloat16

# ---------------- problem constants (hardcoded; kernel.py is self-contained)
N_NODES = 100000
N_EDGES = 1600000
IN_F = 256
OUT_F = 128
NCORES = 8

D_PER_CORE = N_NODES // NCORES  # 12500 dest nodes per core

# launch-1 (support matmul) geometry
ROWS_PAD = 12544  # 98 * 128
RTILES = ROWS_PAD // 128

# launch-2 (streamed halo spmm) geometry
W_G = 32  # dests per bin
CAP = 512  # table rows per bin (4 chunks of 128), one row per edge
EDGE_CAP = CAP  # <=512 edges per bin
CPB = CAP // 128  # chunks per bin = 4
SLAB_CHUNKS = 128  # chunks per slab (32 bins)
BINS_PER_SLAB = SLAB_CHUNKS // CPB  # 32
NSLABS = 13
NBINS = NSLABS * BINS_PER_SLAB  # 416
NCHUNKS = NBINS * CPB  # 1728

FP32 = mybir.dt.float32
BF16 = mybir.dt.bfloat16


def _new_nc():
    return bacc.Bacc("TRN2", target_bir_lowering=False, debug=False)


# ---------------- launch 1: support^T = (X_shard @ W)^T (bf16) ----------------
# Weights stationary (lhsT = W chunk), X columns stream (N=448 per matmul).
L1_N = 448
L1_TILES = ROWS_PAD // L1_N  # 28
L1_GRP = 4  # psum tiles in flight per k-sweep


def build_support_program():
    nc = _new_nc()
    xt = nc.declare_dram_parameter("xt", [2, 128, ROWS_PAD], BF16, isOutput=False)
    w = nc.declare_dram_parameter("w", [2, 128, OUT_F], BF16, isOutput=False)
    sup = nc.declare_dram_parameter("sup", [128, ROWS_PAD], BF16, isOutput=True)

    with tile.TileContext(nc) as tc:
        with (
            tc.tile_pool(name="xt_pool", bufs=1) as xt_pool,
            tc.tile_pool(name="w_pool", bufs=1) as w_pool,
            tc.tile_pool(name="out_pool", bufs=1) as out_pool,
            tc.tile_pool(name="ps_pool", bufs=2, space="PSUM") as ps_pool,
        ):
            w_t = w_pool.tile([128, 2, OUT_F], BF16)
            for k in range(2):
                nc.sync.dma_start(w_t[:, k, :], w[k])
            xt_t = xt_pool.tile([128, 2, ROWS_PAD], BF16)
            piece = L1_N * L1_GRP  # one group's worth of columns
            for h in range(ROWS_PAD // piece):
                for k in range(2):
                    eng = nc.sync if k == 0 else nc.scalar
                    eng.dma_start(
                        xt_t[:, k, piece * h : piece * (h + 1)],
                        xt[k, :, piece * h : piece * (h + 1)],
                    )

            sup_buf = out_pool.tile([128, ROWS_PAD], BF16)
            ngrp = L1_TILES // L1_GRP
            gcols = L1_N * L1_GRP
            for g in range(ngrp):
                pss = [
                    ps_pool.tile([128, L1_N], FP32, space="PSUM", name=f"ps{t}")
                    for t in range(L1_GRP)
                ]
                for k in range(2):
                    for t in range(L1_GRP):
                        i = g * L1_GRP + t
                        nc.tensor.matmul(
                            out=pss[t][:],
                            lhsT=w_t[:, k, :],
                            rhs=xt_t[:, k, L1_N * i : L1_N * (i + 1)],
                            start=(k == 0),
                            stop=(k == 1),
                        )
                for t in range(L1_GRP):
                    i = g * L1_GRP + t
                    nc.vector.tensor_copy(
                        sup_buf[:, L1_N * i : L1_N * (i + 1)], pss[t][:]
                    )
                nc.scalar.dma_start(
                    sup[:, gcols * g : gcols * (g + 1)],
                    sup_buf[:, gcols * g : gcols * (g + 1)],
                )
    nc.compile()
    return nc


# ---------------- launch 2: streamed halo spmm ----------------
def build_spmm_program(real_chunks=NCHUNKS):
    nc = _new_nc()
    # per slab: [w: SLAB_CHUNKS cols][v: SLAB_CHUNKS cols][T: SLAB_CHUNKS*OUT_F]
    VOFF = SLAB_CHUNKS
    TOFF = 2 * SLAB_CHUNKS
    SCOLS = SLAB_CHUNKS * (OUT_F + 2)
    stream = nc.declare_dram_parameter(
        "stream", [NSLABS, 128, SCOLS], BF16, isOutput=False
    )
    iota = nc.declare_dram_parameter("iota", [128, 1, W_G], BF16, isOutput=False)
    out = nc.declare_dram_parameter(
        "out", [NSLABS, W_G, BINS_PER_SLAB * OUT_F], BF16, isOutput=True
    )

    groups_per_slab = BINS_PER_SLAB // 4  # 4 bins per PSUM bank
    NQ = 4  # T quarter-loads per slab
    QC = SLAB_CHUNKS // NQ  # chunks per quarter

    with tile.TileContext(nc) as tc:
        with (
            tc.tile_pool(name="const_pool", bufs=1) as const_pool,
            tc.tile_pool(name="s_pool", bufs=3) as s_pool,
            tc.tile_pool(name="c_pool", bufs=2) as c_pool,
            tc.tile_pool(name="o_pool", bufs=2) as o_pool,
            tc.tile_pool(name="ps_pool", bufs=4, space="PSUM") as ps_pool,
        ):
            iota_t = const_pool.tile([128, 1, W_G], BF16)
            nc.sync.dma_start(iota_t[:], iota[:])

            for s in range(NSLABS):
                # chunks of this slab that hold real bins (rest skipped)
                live = min(max(real_chunks - s * SLAB_CHUNKS, 0), SLAB_CHUNKS)
                if live == 0:
                    break
                st = s_pool.tile([128, SCOLS], BF16)
                nc.sync.dma_start(st[:, :TOFF], stream[s, :, :TOFF])  # w|v first
                cb = c_pool.tile([128, SLAB_CHUNKS, W_G], BF16)
                ceq = c_pool.tile([128, SLAB_CHUNKS, W_G], BF16)
                for qi in range(NQ):
                    lo, hi = QC * qi, QC * (qi + 1)
                    if lo * OUT_F >= live * OUT_F:
                        break
                    tl = min(hi, live)
                    nc.sync.dma_start(
                        st[:, TOFF + OUT_F * lo : TOFF + OUT_F * tl],
                        stream[s, :, TOFF + OUT_F * lo : TOFF + OUT_F * tl],
                    )
                    # build C for this quarter: cb = (w == iota) * v
                    nc.vector.tensor_tensor(
                        out=ceq[:, lo:hi, :],
                        in0=st[:, lo:hi].to_broadcast([128, QC, W_G]),
                        in1=iota_t[:].to_broadcast([128, QC, W_G]),
                        op=mybir.AluOpType.is_equal,
                    )
                    nc.vector.tensor_tensor(
                        out=cb[:, lo:hi, :],
                        in0=ceq[:, lo:hi, :],
                        in1=st[:, VOFF + lo : VOFF + hi].to_broadcast([128, QC, W_G]),
                        op=mybir.AluOpType.mult,
                    )

                o_t = o_pool.tile([W_G, BINS_PER_SLAB * OUT_F], BF16)
                live_groups = (live + 4 * CPB - 1) // (4 * CPB)
                for g in range(live_groups):
                    ps = ps_pool.tile([W_G, 4 * OUT_F], FP32, space="PSUM")
                    for j in range(4):  # bin within group
                        b = g * 4 + j
                        if b * CPB >= live:
                            break
                        for k in range(CPB):
                            c = b * CPB + k
                            nc.tensor.matmul(
                                out=ps[:, OUT_F * j : OUT_F * (j + 1)],
                                lhsT=cb[:, c, :],
                                rhs=st[:, TOFF + OUT_F * c : TOFF + OUT_F * (c + 1)],
                                start=(k == 0),
                                stop=(k == CPB - 1),
                            )
                    nc.scalar.copy(
                        out=o_t[:, 4 * OUT_F * g : 4 * OUT_F * (g + 1)],
                        in_=ps[:],
                    )
                nc.scalar.dma_start(
                    out[s, :, : 4 * OUT_F * live_groups],
                    o_t[:, : 4 * OUT_F * live_groups],
                )
    nc.compile()
    return nc


# ---------------- host-side packing ----------------
def _pack_core(rows_c, cols_c, vals_c, support_bf):
    """Pack one core's edges into (tswz, cswz, destmap).

    rows_c: local dest ids [0, 12500); cols_c: global src ids; vals_c: f32.
    Returns stream [NSLABS,128,SLAB_CHUNKS*(OUT_F+2)] bf16 ([w|v|T] per
    slab) and destmap [NBINS*W_G] int64 (-1 for unused slots, multiple
    slots may map to one dest -- host sums, then de-dupes bias).
    """
    deg = np.bincount(rows_c, minlength=D_PER_CORE)

    # balanced two-pointer binning with dest splitting: <=32 slots and
    # <=EDGE_CAP edges per bin. Take from the high-degree end when the
    # remaining capacity-per-slot exceeds the average degree, else from
    # the low end; a dest whose edges overflow the bin is split across
    # bins (host sums the partial outputs; bias counted once).
    order = np.argsort(-deg, kind="stable")
    degs = deg[order].astype(np.int64)
    n = len(order)
    avg = degs.sum() / D_PER_CORE
    piece_dest, piece_bin, piece_w, piece_take, piece_first = [], [], [], [], []
    i, j = 0, n - 1
    rem_front = int(degs[0])
    front_first = True
    b = 0

    def place(d, w, take, first):
        piece_dest.append(d)
        piece_bin.append(b)
        piece_w.append(w)
        piece_take.append(take)
        piece_first.append(first)

    while i <= j:
        slots, fill = 0, 0
        while slots < W_G and i <= j:
            cap = EDGE_CAP - fill
            if i == j:
                take = min(rem_front, cap)
                if take == 0 and rem_front > 0:
                    break
                place(int(order[i]), slots, take, front_first)
                front_first = False
                slots += 1
                fill += take
                rem_front -= take
                if rem_front == 0:
                    i += 1
                continue
            if (cap / (W_G - slots)) >= avg:
                take = min(rem_front, cap)
                if take < rem_front and take == 0:
                    break
                place(int(order[i]), slots, take, front_first)
                front_first = False
                slots += 1
                fill += take
                rem_front -= take
                if rem_front == 0:
                    i += 1
                    rem_front = int(degs[i]) if i < n else 0
                    front_first = True
            else:
                db = int(degs[j])
                if db <= cap:
                    place(int(order[j]), slots, db, True)
                    slots += 1
                    fill += db
                    j -= 1
                else:
                    if cap == 0:
                        break
                    take = min(rem_front, cap)
                    place(int(order[i]), slots, take, front_first)
                    front_first = False
                    slots += 1
                    fill += take
                    rem_front -= take
                    if rem_front == 0:
                        i += 1
                        rem_front = int(degs[i]) if i < n else 0
                        front_first = True
        b += 1
    nbins_used = b
    if nbins_used > NBINS:
        raise RuntimeError(f"bin overflow: {nbins_used} > {NBINS}")
    piece_dest = np.array(piece_dest, np.int64)
    piece_bin = np.array(piece_bin, np.int64)
    piece_w = np.array(piece_w, np.int64)
    piece_take = np.array(piece_take, np.int64)
    piece_first = np.array(piece_first, bool)

    destmap = np.full(NBINS * W_G, -1, np.int64)
    destmap[piece_bin * W_G + piece_w] = piece_dest

    # per-edge piece: edges sorted by dest; rank within dest selects piece
    order_d = np.argsort(rows_c, kind="stable")
    dstart = np.zeros(D_PER_CORE + 1, np.int64)
    np.cumsum(deg, out=dstart[1:])
    rank = np.arange(len(rows_c)) - dstart[rows_c[order_d]]
    # piece boundaries per dest: order pieces by (dest, first-come)
    po = np.lexsort((np.arange(len(piece_dest)), piece_dest))
    p_d = piece_dest[po]
    p_take = piece_take[po]
    p_off = np.zeros(len(po), np.int64)
    newd = np.empty(len(po), bool)
    newd[0] = True
    np.not_equal(p_d[1:], p_d[:-1], out=newd[1:])
    csum = np.cumsum(p_take) - p_take
    base = np.where(newd, csum, 0)
    np.maximum.accumulate(base, out=base)
    p_off = csum - base  # start rank of each piece within its dest
    # map each edge (dest, rank) -> piece index via searchsorted per dest
    pstart_of_dest = np.zeros(D_PER_CORE + 1, np.int64)
    np.cumsum(np.bincount(p_d, minlength=D_PER_CORE), out=pstart_of_dest[1:])
    ed = rows_c[order_d]
    lo = pstart_of_dest[ed]
    hi = pstart_of_dest[ed + 1]
    # pieces per dest are tiny (1-2); resolve by comparing rank to offsets
    pidx = lo.copy()
    multi = hi - lo > 1
    if multi.any():
        # iterate piece levels (max pieces per dest is small)
        maxp = int((hi - lo).max())
        for lvl in range(1, maxp):
            cand = lo + lvl
            ok = (cand < hi) & (rank >= p_off[np.minimum(cand, len(p_off) - 1)])
            pidx = np.where(ok, cand, pidx)
    e_bin = np.empty(len(rows_c), np.int64)
    e_w = np.empty(len(rows_c), np.int64)
    e_bin[order_d] = piece_bin[po][pidx]
    e_w[order_d] = piece_w[po][pidx]

    # one table slot per edge: sort edges by bin, slot = rank within bin
    order_e = np.argsort(e_bin, kind="stable")
    eb = e_bin[order_e]
    ec = cols_c[order_e]
    ew = e_w[order_e]
    ev = vals_c[order_e]
    bin_start = np.zeros(nbins_used + 1, np.int64)
    np.cumsum(np.bincount(eb, minlength=nbins_used), out=bin_start[1:])
    e_slot = np.arange(len(eb)) - bin_start[eb]
    if len(e_slot) and e_slot.max() >= CAP:
        raise RuntimeError("edge overflow in a bin")

    rows_idx = eb * CAP + e_slot
    tidx = np.zeros(NBINS * CAP, np.int64)
    tidx[rows_idx] = ec
    t_all = support_bf[tidx]  # [NBINS*CAP, OUT_F] bf16 (pad rows killed by v=0)
    w_all = np.zeros(NBINS * CAP, np.float32)
    w_all[rows_idx] = ew
    v_all = np.zeros(NBINS * CAP, np.float32)
    v_all[rows_idx] = ev

    # swizzle: chunk rows -> partitions; stream = [w | v | T] per slab
    wswz = w_all.astype(BF16_NP).reshape(NSLABS, SLAB_CHUNKS, 128).transpose(0, 2, 1)
    vswz = v_all.astype(BF16_NP).reshape(NSLABS, SLAB_CHUNKS, 128).transpose(0, 2, 1)
    tswz = (
        t_all.reshape(NSLABS, SLAB_CHUNKS, 128, OUT_F)
        .transpose(0, 2, 1, 3)
        .reshape(NSLABS, 128, SLAB_CHUNKS * OUT_F)
    )
    stream = np.ascontiguousarray(np.concatenate([wswz, vswz, tswz], axis=2))
    return stream, destmap


def kernel(X_input, adj_row, adj_col, adj_val, W, bias):
    X_input = np.asarray(X_input, np.float32)
    adj_row = np.asarray(adj_row).astype(np.int64)
    adj_col = np.asarray(adj_col).astype(np.int64)
    adj_val = np.asarray(adj_val, np.float32)
    W = np.asarray(W, np.float32)
    bias = np.asarray(bias, np.float32)

    # ---- launch 1: support shards (bf16)
    nc1 = build_support_program()
    w_bf = np.ascontiguousarray(W.astype(BF16_NP).reshape(2, 128, OUT_F))
    in_maps1 = []
    for c in range(NCORES):
        sl = np.zeros((ROWS_PAD, IN_F), np.float32)
        lo = c * D_PER_CORE
        sl[:D_PER_CORE] = X_input[lo : lo + D_PER_CORE]
        xt = np.ascontiguousarray(
            sl.T.astype(BF16_NP).reshape(2, 128, ROWS_PAD)
        )
        in_maps1.append({"xt": xt, "w": w_bf})
    res1 = run_bass_kernel_spmd(nc1, in_maps1, list(range(NCORES)))
    kernel.last_res1 = res1
    shards = []
    for c in range(NCORES):
        s = res1.results[c]["sup"]  # [128, ROWS_PAD] bf16 = support^T
        shards.append(s.T[:D_PER_CORE])
    support_bf = np.ascontiguousarray(np.concatenate(shards, axis=0)).astype(BF16_NP)

    # ---- host packing
    iota_arr = np.ascontiguousarray(
        np.broadcast_to(np.arange(W_G, dtype=np.float32), (128, 1, W_G))
    ).astype(BF16_NP)
    core_of = adj_row // D_PER_CORE
    in_maps2 = []
    destmaps = []
    for c in range(NCORES):
        m = core_of == c
        stream, destmap = _pack_core(
            adj_row[m] - c * D_PER_CORE,
            adj_col[m],
            adj_val[m],
            support_bf,
        )
        destmaps.append(destmap)
        in_maps2.append({"stream": stream, "iota": iota_arr})

    # ---- launch 2 (compiled for the worst-case real bin count)
    maxbins = max(
        int((dm.reshape(NBINS, W_G) >= 0).any(axis=1).sum()) for dm in destmaps
    )
    nc2 = build_spmm_program(real_chunks=maxbins * CPB)
    res2 = run_bass_kernel_spmd(nc2, in_maps2, list(range(NCORES)))
    kernel.last_res2 = res2
    out = np.empty((N_NODES, OUT_F), np.float32)
    for c in range(NCORES):
        o = res2.results[c]["out"]  # [NSLABS, W_G, BINS_PER_SLAB*OUT_F] bf16
        # slot (bin, w) -> o[s, w, bi*OUT_F : ...] where bin = s*BINS_PER_SLAB+bi
        o = (
            o.reshape(NSLABS, W_G, BINS_PER_SLAB, OUT_F)
            .transpose(0, 2, 1, 3)
            .reshape(NBINS * W_G, OUT_F)
        )
        dm = destmaps[c]
        valid = dm >= 0
        shard = np.zeros((D_PER_CORE, OUT_F), np.float32)
        np.add.at(shard, dm[valid], o[valid].astype(np.float32))
        shard += bias
        out[c * D_PER_CORE : (c + 1) * D_PER_CORE] = shard
    return out
